# revision 3
# baseline (speedup 1.0000x reference)
"""DCNv2 (modulated deformable convolution) on 8 Trainium2 NeuronCores.

kernel(**inputs) takes the full unsharded inputs
    x      (8, 128, 64, 64) f32
    w_om   (27, 128, 3, 3)  f32
    b_om   (27,)            f32
    weight (128, 128, 3, 3) f32
    bias   (128,)           f32
and returns the full output (8, 128, 64, 64) f32.

Sharding: pure data-parallel over batch — one image per NeuronCore, small
weights replicated; no collectives.

Per-core Bass/Tile program:
  1. offset conv (27ch 3x3) on the PE as 9 shifted matmuls over padded x
  2. softmax mask + bilinear coefficients + gather indices on DVE/ACT,
     batched over the whole image in a (pixel-partition, (tile,tap)-free)
     layout
  3. a zero-padded channel-transposed image xT (72x72 rows x 128ch) is
     staged in DRAM; per tap two dma_gather calls fetch the 4 bilinear
     corners as row pairs into (pixel, channel) tiles
  4. corners are combined with per-partition-scalar MACs
     (tensor_scalar + scalar_tensor_tensor), PE-transposed back to
     (channel, pixel) and accumulated over the 9 taps into PSUM with the
     128x128x3x3 weight; bias is added on the PSUM->SBUF copy.
"""

import os
import sys

import numpy as np

sys.path.insert(0, "/opt/trn_rl_repo")

from contextlib import ExitStack

import concourse.bacc as bacc
import concourse.mybir as mybir
import concourse.tile as tile
from concourse._compat import get_trn_type
from concourse.alu_op_type import AluOpType as Alu
from concourse.bass import AP
from concourse.bass_utils import run_bass_kernel_spmd
from concourse import library_config

F32 = mybir.dt.float32
I32 = mybir.dt.int32
I16 = mybir.dt.int16

B = 8
C = 128
H = W = 64
HW = H * W
K2 = 9
PADG = 4
GW = H + 2 * PADG      # 72
GROWS = GW * GW        # 5184
NS = 32
NHALF = 2
S_PER_HALF = NS // NHALF
PIX_PER_HALF = HW // NHALF

LAST_EXEC_TIME_NS = None
LAST_RESULT = None


def _emit(tc):
    nc = tc.nc
    x_d = nc.dram_tensor("x", [C, HW], F32, kind="ExternalInput").ap()
    w_om_d = nc.dram_tensor("w_om", [27, 1152], F32, kind="ExternalInput").ap()
    b_om_d = nc.dram_tensor("b_om", [27, 1], F32, kind="ExternalInput").ap()
    weight_d = nc.dram_tensor("weight", [C, 1152], F32, kind="ExternalInput").ap()
    bias_d = nc.dram_tensor("bias", [C, 1], F32, kind="ExternalInput").ap()
    out_d = nc.dram_tensor("out", [C, HW], F32, kind="ExternalOutput").ap()
    xt_d = nc.dram_tensor("xt_pad", [GROWS, C], F32, kind="ExternalOutput").ap()
    consts_d = nc.dram_tensor("consts", [128, 707], F32, kind="ExternalInput").ap()

    nc.gpsimd.load_library(library_config.mlp)

    ctx = ExitStack()
    with ctx:
        cpool = ctx.enter_context(tc.tile_pool(name="const", bufs=1))
        spool = ctx.enter_context(tc.tile_pool(name="setup", bufs=1))
        dpool = ctx.enter_context(tc.tile_pool(name="data", bufs=1))
        gpool = ctx.enter_context(tc.tile_pool(name="gath", bufs=2))
        vpool = ctx.enter_context(tc.tile_pool(name="val", bufs=3))
        ppool = ctx.enter_context(tc.tile_pool(name="psum", bufs=1, space="PSUM"))
        tpool = ctx.enter_context(tc.tile_pool(name="trps", bufs=2, space="PSUM"))
        opool = ctx.enter_context(tc.tile_pool(name="omps", bufs=2, space="PSUM"))

        # ---------- constants (host-provided; gpsimd iota lives in a
        # different loadable Q7 library than dma_gather) ----------
        cons = cpool.tile([128, 707], F32)
        nc.sync.dma_start(cons[:], consts_d[:, :])
        ident = cons[:, 0:128]
        hob = cons[:, 129:130]
        wo_r = cons[:, 130:131]
        ykc = cons[:, 131:419]
        xkc = cons[:, 419:707]

        # ---------- load x; build xT_pad in DRAM ----------
        x_sb = spool.tile([128, HW], F32)
        nc.sync.dma_start(x_sb[:], x_d[:, :])

        zt = spool.tile([128, 648], F32)
        nc.vector.memset(zt[:], 0.0)
        for i in range(8):
            nc.sync.dma_start(
                AP(xt_d.tensor, i * 128 * 648, [[648, 128], [1, 648]]), zt[:]
            )
        for s in range(NS):
            trp = tpool.tile([128, 128], F32, tag="tr", name="trp")
            nc.tensor.transpose(trp[:], x_sb[:, s * 128:(s + 1) * 128], ident)
            stg = vpool.tile([128, 128], F32, tag="xtstage", name="stg")
            nc.scalar.copy(stg[:], trp[:])
            dst = AP(
                xt_d.tensor,
                ((2 * s + PADG) * GW + PADG) * 128,
                [[GW * 128, 2], [128, 64], [1, 128]],
            )
            nc.sync.dma_start(dst, stg[:])

        # ---------- x_pad for the offset conv ----------
        XP = 66
        x_pad = spool.tile([128, XP * XP], F32)
        nc.vector.memset(x_pad[:], 0.0)
        nc.sync.dma_start(
            x_pad[:].rearrange("p (a b) -> p a b", a=XP)[:, 1:65, 1:65], x_d[:, :]
        )

        # ---------- weights ----------
        w_om_sb = spool.tile([27, 1152], F32)
        nc.sync.dma_start(w_om_sb[:], w_om_d[:, :])
        b_om_sb = spool.tile([27, 1], F32)
        nc.sync.dma_start(b_om_sb[:], b_om_d[:, :])
        weight_sb = spool.tile([128, 1152], F32)
        nc.sync.dma_start(weight_sb[:], weight_d[:, :])
        bias_sb = spool.tile([128, 1], F32)
        nc.sync.dma_start(bias_sb[:], bias_d[:, :])

        wT = spool.tile([128, K2, 128], F32)
        for k in range(K2):
            trp = tpool.tile([128, 128], F32, tag="tr", name="trp")
            nc.tensor.transpose(
                trp[:], weight_sb[:].rearrange("p (c k) -> p c k", k=K2)[:, :, k],
                ident,
            )
            nc.scalar.copy(wT[:, k, :], trp[:])
        womT = spool.tile([128, K2, 27], F32)
        for k in range(K2):
            trp = tpool.tile([128, 128], F32, tag="tr", name="trp")
            nc.tensor.transpose(
                trp[:, :27], w_om_sb[:].rearrange("p (c k) -> p c k", k=K2)[:, :, k],
                ident[0:27, 0:27],
            )
            nc.scalar.copy(womT[:, k, :], trp[:, :27])

        # ---------- offset conv: om (27, 4096) ----------
        om_sb = spool.tile([27, HW], F32)
        xpv = x_pad[:].rearrange("p (a b) -> p a b", a=XP)
        for ch in range(8):
            omp = opool.tile([27, 512], F32, tag="om", name="omp")
            for k in range(K2):
                dy_, dx_ = k // 3, k % 3
                r0 = ch * 8 + dy_
                nc.tensor.matmul(
                    omp[:], womT[:, k, :27], xpv[:, r0:r0 + 8, dx_:dx_ + 64],
                    start=(k == 0), stop=(k == K2 - 1),
                )
            nc.scalar.activation(
                om_sb[:, ch * 512:(ch + 1) * 512], omp[:],
                mybir.ActivationFunctionType.Identity, bias=b_om_sb[:], scale=1.0,
            )

        # ---------- omT (128 pix, 27) per s-tile ----------
        omT = spool.tile([128, NS, 27], F32)
        for s in range(NS):
            trp = tpool.tile([128, 128], F32, tag="tr", name="trp")
            nc.tensor.transpose(
                trp[:, :27], om_sb[:, s * 128:(s + 1) * 128], ident[0:27, 0:27]
            )
            nc.scalar.copy(omT[:, s, :], trp[:, :27])

        # ---------- coefficient pipeline (128, 32, 9) ----------
        _cnt = [0]

        def f(shape=(128, NS, K2), dt=F32, tag=None):
            _cnt[0] += 1
            nm = f"cf{_cnt[0]}"
            return dpool.tile(list(shape), dt, tag=tag or nm, name=nm)

        # offset channel layout: dy_k = om[2k], dx_k = om[2k+1]
        omT_t = omT[:].tensor
        omT_off = omT[:].offset
        dyT = AP(omT_t, omT_off + 0, [[NS * 27, 128], [27, NS], [2, K2]])
        dxT = AP(omT_t, omT_off + 1, [[NS * 27, 128], [27, NS], [2, K2]])
        mlg = omT[:, :, 18:27]

        e = f()
        nc.scalar.activation(e[:], mlg, mybir.ActivationFunctionType.Exp)
        ssum = f((128, NS, 1))
        nc.vector.tensor_reduce(ssum[:], e[:], mybir.AxisListType.X, Alu.add)
        rs = f((128, NS, 1))
        nc.vector.reciprocal(rs[:], ssum[:])
        mask = f()
        nc.vector.tensor_tensor(mask[:], e[:], rs[:].to_broadcast([128, NS, K2]),
                                Alu.mult)

        ykv = ykc.rearrange("p (s a) -> p s a", a=K2)
        xkv = xkc.rearrange("p (s a) -> p s a", a=K2)
        py = f()
        nc.vector.scalar_tensor_tensor(py[:], dyT, hob, ykv, Alu.add, Alu.add)
        px = f()
        nc.vector.scalar_tensor_tensor(px[:], dxT, wo_r, xkv, Alu.add, Alu.add)

        def floorit(v):
            vi = f(dt=I32, tag="fl_i")
            nc.vector.tensor_copy(vi[:], v[:])
            v0 = f(tag="fl_f")
            nc.vector.tensor_copy(v0[:], vi[:])
            gt = f(tag="fl_gt")
            nc.vector.tensor_tensor(gt[:], v0[:], v[:], Alu.is_gt)
            v0f = f()
            nc.vector.tensor_tensor(v0f[:], v0[:], gt[:], Alu.subtract)
            return v0f

        y0f = floorit(py)
        x0f = floorit(px)
        wy1 = f()
        nc.vector.tensor_tensor(wy1[:], py[:], y0f[:], Alu.subtract)
        wy0 = f()
        nc.vector.tensor_scalar(wy0[:], wy1[:], -1.0, 1.0, Alu.mult, Alu.add)
        wx1 = f()
        nc.vector.tensor_tensor(wx1[:], px[:], x0f[:], Alu.subtract)
        wx0 = f()
        nc.vector.tensor_scalar(wx0[:], wx1[:], -1.0, 1.0, Alu.mult, Alu.add)
        nc.vector.tensor_scalar(y0f[:], y0f[:], -float(PADG), float(H + 2),
                                Alu.max, Alu.min)
        nc.vector.tensor_scalar(x0f[:], x0f[:], -float(PADG), float(W + 2),
                                Alu.max, Alu.min)

        mwy0 = f()
        nc.vector.tensor_tensor(mwy0[:], mask[:], wy0[:], Alu.mult)
        mwy1 = f()
        nc.vector.tensor_tensor(mwy1[:], mask[:], wy1[:], Alu.mult)
        c00 = f()
        nc.vector.tensor_tensor(c00[:], mwy0[:], wx0[:], Alu.mult)
        c01 = f()
        nc.vector.tensor_tensor(c01[:], mwy0[:], wx1[:], Alu.mult)
        c10 = f()
        nc.vector.tensor_tensor(c10[:], mwy1[:], wx0[:], Alu.mult)
        c11 = f()
        nc.vector.tensor_tensor(c11[:], mwy1[:], wx1[:], Alu.mult)

        gAf = f()
        nc.vector.tensor_scalar(gAf[:], y0f[:], float(GW), float(PADG * GW + PADG),
                                Alu.mult, Alu.add)
        nc.vector.tensor_tensor(gAf[:], gAf[:], x0f[:], Alu.add)
        gBf = f()
        nc.vector.tensor_scalar(gBf[:], gAf[:], float(GW), None, Alu.add)

        idxA = dpool.tile([128, K2, NS], I16)
        idxB = dpool.tile([128, K2, NS], I16)
        idxA_w = AP(idxA[:].tensor, 0, [[K2 * NS, 128], [1, NS], [NS, K2]])
        idxB_w = AP(idxB[:].tensor, 0, [[K2 * NS, 128], [1, NS], [NS, K2]])
        nc.vector.tensor_copy(idxA_w, gAf[:])
        nc.vector.tensor_copy(idxB_w, gBf[:])

        # ---------- idx re-wrap to dma_gather layout ----------
        idxAw = dpool.tile([128, K2 * 256], I16)
        idxBw = dpool.tile([128, K2 * 256], I16)
        for (src, dst) in ((idxA, idxAw), (idxB, idxBw)):
            for u in range(8):
                d_ap = AP(dst[:].tensor, u, [[K2 * 256, 16], [256, K2], [8, NS]])
                nc.sync.dma_start(d_ap, src[16 * u:16 * u + 16, :, :])
            for u in range(1, 8):
                nc.sync.dma_start(dst[16 * u:16 * u + 16, :], dst[0:16, :])

        # ---------- main loop ----------
        out_sb = dpool.tile([128, HW], F32)
        xt_src = AP(xt_d.tensor, 0, [[128, GROWS - 1], [1, 256]])
        for h in range(NHALF):
            outp = ppool.tile([128, PIX_PER_HALF], F32, tag="out", name="outp")
            for k in range(K2):
                gA = gpool.tile([128, S_PER_HALF, 256], F32, tag="gA", name="gA")
                gB = gpool.tile([128, S_PER_HALF, 256], F32, tag="gB", name="gB")
                nc.gpsimd.dma_gather(
                    gA[:], xt_src,
                    idxAw[:, k * 256 + 128 * h: k * 256 + 128 * h + 128],
                    PIX_PER_HALF, PIX_PER_HALF, 256, elem_step=128,
                    single_packet=False,
                )
                nc.gpsimd.dma_gather(
                    gB[:], xt_src,
                    idxBw[:, k * 256 + 128 * h: k * 256 + 128 * h + 128],
                    PIX_PER_HALF, PIX_PER_HALF, 256, elem_step=128,
                    single_packet=False,
                )
                vT = None
                for t in range(S_PER_HALF):
                    s = h * S_PER_HALF + t
                    v = vpool.tile([128, 128], F32, tag="v", name="v")
                    nc.vector.tensor_scalar_mul(v[:], gA[:, t, 0:128],
                                                c00[:, s, k:k + 1])
                    nc.vector.scalar_tensor_tensor(
                        v[:], gA[:, t, 128:256], c01[:, s, k:k + 1], v[:],
                        Alu.mult, Alu.add)
                    nc.vector.scalar_tensor_tensor(
                        v[:], gB[:, t, 0:128], c10[:, s, k:k + 1], v[:],
                        Alu.mult, Alu.add)
                    nc.vector.scalar_tensor_tensor(
                        v[:], gB[:, t, 128:256], c11[:, s, k:k + 1], v[:],
                        Alu.mult, Alu.add)
                    if t % 4 == 0:
                        vT = vpool.tile([128, 512], F32, tag="vT", name="vT")
                    trp = tpool.tile([128, 128], F32, tag="tr", name="trp")
                    nc.tensor.transpose(trp[:], v[:], ident)
                    nc.scalar.copy(vT[:, (t % 4) * 128:(t % 4) * 128 + 128], trp[:])
                    if t % 4 == 3:
                        bk = t // 4
                        nc.tensor.matmul(
                            outp[:, bk * 512:(bk + 1) * 512], wT[:, k, :], vT[:],
                            start=(k == 0), stop=(k == K2 - 1),
                        )
            for bk in range(4):
                nc.scalar.activation(
                    out_sb[:, h * PIX_PER_HALF + bk * 512:
                           h * PIX_PER_HALF + (bk + 1) * 512],
                    outp[:, bk * 512:(bk + 1) * 512],
                    mybir.ActivationFunctionType.Identity, bias=bias_sb[:],
                    scale=1.0,
                )
        nc.sync.dma_start(out_d[:, :], out_sb[:])


def _make_consts():
    c = np.zeros((128, 707), np.float32)
    c[:, 0:128] = np.eye(128, dtype=np.float32)
    p = np.arange(128)
    c[:, 128] = p
    c[:, 129] = (p >= 64)
    c[:, 130] = p % 64
    s = np.arange(32)[:, None, None]
    kyv = np.arange(3)[None, :, None]
    kxv = np.arange(3)[None, None, :]
    c[:, 131:419] = np.broadcast_to(
        (2 * s + kyv - 1 + 0 * kxv).reshape(-1), (128, 288))
    c[:, 419:707] = np.broadcast_to(
        (0 * s + 0 * kyv + kxv - 1).reshape(-1), (128, 288))
    return c


_COMPILED = None


def _get_compiled():
    global _COMPILED
    if _COMPILED is None:
        nc = bacc.Bacc(get_trn_type() or "TRN2", target_bir_lowering=False,
                       debug=False, num_devices=B)
        with tile.TileContext(nc) as tc:
            _emit(tc)
        nc.compile()
        _COMPILED = nc
    return _COMPILED


def kernel(x, w_om, b_om, weight, bias):
    global LAST_EXEC_TIME_NS
    x = np.ascontiguousarray(np.asarray(x, dtype=np.float32))
    w_om_f = np.ascontiguousarray(np.asarray(w_om, np.float32).reshape(27, 1152))
    b_om_f = np.ascontiguousarray(np.asarray(b_om, np.float32).reshape(27, 1))
    weight_f = np.ascontiguousarray(np.asarray(weight, np.float32).reshape(128, 1152))
    bias_f = np.ascontiguousarray(np.asarray(bias, np.float32).reshape(128, 1))

    nc = _get_compiled()
    consts = _make_consts()
    in_maps = [
        {
            "x": np.ascontiguousarray(x[b].reshape(C, HW)),
            "w_om": w_om_f,
            "b_om": b_om_f,
            "weight": weight_f,
            "bias": bias_f,
            "consts": consts,
        }
        for b in range(B)
    ]
    trace = bool(os.environ.get("DCN_TRACE"))
    res = run_bass_kernel_spmd(nc, in_maps, core_ids=list(range(B)), trace=trace)
    global LAST_RESULT
    LAST_RESULT = res
    LAST_EXEC_TIME_NS = res.exec_time_ns
    out = np.stack([res.results[b]["out"].reshape(C, H, W) for b in range(B)])
    return out.astype(np.float32)



# revision 5
# speedup vs baseline: 1.8989x; 1.8989x over previous
"""DCNv2 (modulated deformable convolution) on 8 Trainium2 NeuronCores.

kernel(**inputs) takes the full unsharded inputs
    x      (8, 128, 64, 64) f32
    w_om   (27, 128, 3, 3)  f32
    b_om   (27,)            f32
    weight (128, 128, 3, 3) f32
    bias   (128,)           f32
and returns the full output (8, 128, 64, 64) f32.

Sharding: pure data-parallel over batch - one image per NeuronCore, small
weights replicated; no collectives.

v2 per-core program (bf16 datapath):
  1. x is cast to bf16 and staged twice into a DRAM image xt2[GROWS, 256]
     where row r=(y,x) holds [C(y,x), C(y+1,x)] - so the 4 bilinear corners
     of any sample are 4*128 CONTIGUOUS bf16 values (one 1KB gather
     descriptor per (pixel, tap) instead of two).
  2. offset conv (27ch 3x3) on the PE in bf16; softmax mask + bilinear
     coefficients in f32 on DVE; gather row-indices are wrapped into the
     16-partition dma_gather layout with PE transposes (no tiny-descriptor
     scatter DMAs).
  3. per (half, tap) one dma_gather fetches [A0 B0 A1 B1] corner blocks in
     (pixel-partition, channel) layout; corners are combined with
     per-partition-scalar MACs split across ACT (1 mul) and DVE (3 fused
     MACs), transposed back to (channel, pixel) on the PE (4 tiles per PSUM
     bank), and accumulated over the 9 taps into PSUM with the 128x128x3x3
     weight; bias is added on the PSUM->SBUF copy.
"""

import os
import sys

import numpy as np

sys.path.insert(0, "/opt/trn_rl_repo")

from contextlib import ExitStack

import concourse.bacc as bacc
import concourse.mybir as mybir
import concourse.tile as tile
from concourse._compat import get_trn_type
from concourse.alu_op_type import AluOpType as Alu
from concourse.bass import AP
from concourse.bass_utils import run_bass_kernel_spmd
from concourse import library_config

F32 = mybir.dt.float32
BF16 = mybir.dt.bfloat16
I32 = mybir.dt.int32
I16 = mybir.dt.int16

B = 8
C = 128
H = W = 64
HW = H * W
K2 = 9
PADG = 4
GW = H + 2 * PADG      # 72
GROWS = GW * GW        # 5184
NS = 32
NHALF = 2
SPH = NS // NHALF      # 16 s-tiles per half
PPH = HW // NHALF      # 2048 pixels per half
IDENT = mybir.ActivationFunctionType.Identity

LAST_EXEC_TIME_NS = None
LAST_RESULT = None


def _emit(tc):
    nc = tc.nc
    x_d = nc.dram_tensor("x", [C, HW], F32, kind="ExternalInput").ap()
    w_om_d = nc.dram_tensor("w_om", [27, 1152], F32, kind="ExternalInput").ap()
    b_om_d = nc.dram_tensor("b_om", [27, 1], F32, kind="ExternalInput").ap()
    weight_d = nc.dram_tensor("weight", [C, 1152], F32, kind="ExternalInput").ap()
    bias_d = nc.dram_tensor("bias", [C, 1], F32, kind="ExternalInput").ap()
    out_d = nc.dram_tensor("out", [C, HW], F32, kind="ExternalOutput").ap()
    xt2_d = nc.dram_tensor("xt2_pad", [GROWS, 256], BF16, kind="Internal").ap()
    consts_d = nc.dram_tensor("consts", [128, 707], F32, kind="ExternalInput").ap()

    nc.gpsimd.load_library(library_config.mlp)

    ctx = ExitStack()
    with ctx:
        cpool = ctx.enter_context(tc.tile_pool(name="const", bufs=1))
        spool = ctx.enter_context(tc.tile_pool(name="setup", bufs=1))
        dpool = ctx.enter_context(tc.tile_pool(name="data", bufs=1))
        gpool = ctx.enter_context(tc.tile_pool(name="gath", bufs=2))
        vpool = ctx.enter_context(tc.tile_pool(name="val", bufs=3))
        ppool = ctx.enter_context(tc.tile_pool(name="psum", bufs=1, space="PSUM"))
        tpool = ctx.enter_context(tc.tile_pool(name="trps", bufs=2, space="PSUM"))
        opool = ctx.enter_context(tc.tile_pool(name="omps", bufs=2, space="PSUM"))

        # ---------- constants ----------
        cons = cpool.tile([128, 707], F32)
        nc.sync.dma_start(cons[:], consts_d[:, :])
        ident = cons[:, 0:128]
        hob = cons[:, 129:130]
        wo_r = cons[:, 130:131]
        ykc = cons[:, 131:419]
        xkc = cons[:, 419:707]
        identb = spool.tile([128, 128], BF16)
        nc.vector.tensor_copy(identb[:], ident)

        # ---------- load x; cast to bf16 ----------
        x_sb = spool.tile([128, HW], F32)
        nc.sync.dma_start(x_sb[:], x_d[:, :])
        x16 = spool.tile([128, HW], BF16)
        nc.vector.tensor_copy(x16[:], x_sb[:])

        # ---------- zero-fill xt2 (2.65 MB bf16) ----------
        zt = spool.tile([128, 1296], BF16)
        nc.vector.memset(zt[:], 0.0)
        for i in range(8):
            nc.sync.dma_start(
                AP(xt2_d.tensor, i * 128 * 1296, [[1296, 128], [1, 1296]]), zt[:]
            )

        # ---------- stage xt2: transpose x16 and write row-pairs ----------
        # tile s holds pixels of image rows y=2s,2s+1 (64 x each) on partitions
        for s in range(NS):
            trp = tpool.tile([128, 512], BF16, tag="tr", name="trp")
            nc.tensor.transpose(trp[:, 0:128], x16[:, s * 128:(s + 1) * 128],
                                identb[:])
            stg = vpool.tile([128, 128], BF16, tag="stg", name="stg")
            nc.scalar.copy(stg[:], trp[:, 0:128])
            # first halves: xt2[(2s+ry+PADG)*GW + PADG + x][0:128] = C(2s+ry, x)
            nc.sync.dma_start(
                AP(xt2_d.tensor, ((2 * s + PADG) * GW + PADG) * 256,
                   [[GW * 256, 2], [256, 64], [1, 128]]),
                stg[:],
            )
            # second halves: xt2[(2s+ry-1+PADG)*GW + PADG + x][128:256] = C(2s+ry, x)
            nc.sync.dma_start(
                AP(xt2_d.tensor, ((2 * s - 1 + PADG) * GW + PADG) * 256 + 128,
                   [[GW * 256, 2], [256, 64], [1, 128]]),
                stg[:],
            )

        # ---------- x_pad (bf16) for the offset conv ----------
        XP = 66
        x_pad = spool.tile([128, XP * XP], BF16)
        nc.vector.memset(x_pad[:], 0.0)
        nc.vector.tensor_copy(
            x_pad[:].rearrange("p (a b) -> p a b", a=XP)[:, 1:65, 1:65],
            x16[:].rearrange("p (a b) -> p a b", a=64),
        )

        # ---------- weights ----------
        w_om_sb = spool.tile([27, 1152], F32)
        nc.sync.dma_start(w_om_sb[:], w_om_d[:, :])
        w_om16 = spool.tile([27, 1152], BF16)
        nc.vector.tensor_copy(w_om16[:], w_om_sb[:])
        b_om_sb = spool.tile([27, 1], F32)
        nc.sync.dma_start(b_om_sb[:], b_om_d[:, :])
        weight_sb = spool.tile([128, 1152], F32)
        nc.sync.dma_start(weight_sb[:], weight_d[:, :])
        w16 = spool.tile([128, 1152], BF16)
        nc.vector.tensor_copy(w16[:], weight_sb[:])
        bias_sb = spool.tile([128, 1], F32)
        nc.sync.dma_start(bias_sb[:], bias_d[:, :])

        wT = spool.tile([128, K2, 128], BF16)
        for k in range(K2):
            trp = tpool.tile([128, 512], BF16, tag="tr", name="trp")
            nc.tensor.transpose(
                trp[:, 0:128],
                w16[:].rearrange("p (c k) -> p c k", k=K2)[:, :, k], identb[:],
            )
            nc.scalar.copy(wT[:, k, :], trp[:, 0:128])
        womT = spool.tile([128, K2, 27], BF16)
        for k in range(K2):
            trp = tpool.tile([128, 512], BF16, tag="tr", name="trp")
            nc.tensor.transpose(
                trp[:, 0:27],
                w_om16[:].rearrange("p (c k) -> p c k", k=K2)[:, :, k],
                identb[0:27, 0:27],
            )
            nc.scalar.copy(womT[:, k, :], trp[:, 0:27])

        # ---------- offset conv: om (27, 4096) f32 ----------
        om_sb = spool.tile([27, HW], F32)
        xpv = x_pad[:].rearrange("p (a b) -> p a b", a=XP)
        for ch in range(8):
            omp = opool.tile([128, 512], F32, tag="om", name="omp")
            for k in range(K2):
                dy_, dx_ = k // 3, k % 3
                r0 = ch * 8 + dy_
                nc.tensor.matmul(
                    omp[0:27, :], womT[:, k, :], xpv[:, r0:r0 + 8, dx_:dx_ + 64],
                    start=(k == 0), stop=(k == K2 - 1),
                )
            nc.scalar.activation(
                om_sb[:, ch * 512:(ch + 1) * 512], omp[0:27, :],
                IDENT, bias=b_om_sb[:], scale=1.0,
            )

        # ---------- omT (128 pix, 27) per s-tile ----------
        omT = spool.tile([128, NS, 27], F32)
        for s in range(NS):
            trp = opool.tile([128, 512], F32, tag="om", name="omp")
            nc.tensor.transpose(
                trp[:, 0:27], om_sb[:, s * 128:(s + 1) * 128], ident[0:27, 0:27]
            )
            nc.scalar.copy(omT[:, s, :], trp[:, 0:27])

        # ---------- coefficient pipeline (128, 32, 9) f32 ----------
        _cnt = [0]

        def f(shape=(128, NS, K2), dt=F32, tag=None):
            _cnt[0] += 1
            nm = f"cf{_cnt[0]}"
            return dpool.tile(list(shape), dt, tag=tag or nm, name=nm)

        omT_t = omT[:].tensor
        omT_off = omT[:].offset
        dyT = AP(omT_t, omT_off + 0, [[NS * 27, 128], [27, NS], [2, K2]])
        dxT = AP(omT_t, omT_off + 1, [[NS * 27, 128], [27, NS], [2, K2]])
        mlg = omT[:, :, 18:27]

        e = f()
        nc.scalar.activation(e[:], mlg, mybir.ActivationFunctionType.Exp)
        ssum = f((128, NS, 1))
        nc.vector.tensor_reduce(ssum[:], e[:], mybir.AxisListType.X, Alu.add)
        rs = f((128, NS, 1))
        nc.vector.reciprocal(rs[:], ssum[:])
        mask = f()
        nc.vector.tensor_tensor(mask[:], e[:], rs[:].to_broadcast([128, NS, K2]),
                                Alu.mult)

        ykv = ykc.rearrange("p (s a) -> p s a", a=K2)
        xkv = xkc.rearrange("p (s a) -> p s a", a=K2)
        py = f()
        nc.vector.scalar_tensor_tensor(py[:], dyT, hob, ykv, Alu.add, Alu.add)
        px = f()
        nc.vector.scalar_tensor_tensor(px[:], dxT, wo_r, xkv, Alu.add, Alu.add)

        def floorit(v):
            vi = f(dt=I32, tag="fl_i")
            nc.vector.tensor_copy(vi[:], v[:])
            v0 = f(tag="fl_f")
            nc.vector.tensor_copy(v0[:], vi[:])
            gt = f(tag="fl_gt")
            nc.vector.tensor_tensor(gt[:], v0[:], v[:], Alu.is_gt)
            v0f = f()
            nc.vector.tensor_tensor(v0f[:], v0[:], gt[:], Alu.subtract)
            return v0f

        y0f = floorit(py)
        x0f = floorit(px)
        wy1 = f()
        nc.vector.tensor_tensor(wy1[:], py[:], y0f[:], Alu.subtract)
        wy0 = f()
        nc.vector.tensor_scalar(wy0[:], wy1[:], -1.0, 1.0, Alu.mult, Alu.add)
        wx1 = f()
        nc.vector.tensor_tensor(wx1[:], px[:], x0f[:], Alu.subtract)
        wx0 = f()
        nc.vector.tensor_scalar(wx0[:], wx1[:], -1.0, 1.0, Alu.mult, Alu.add)
        nc.vector.tensor_scalar(y0f[:], y0f[:], -float(PADG), float(H + 2),
                                Alu.max, Alu.min)
        nc.vector.tensor_scalar(x0f[:], x0f[:], -float(PADG), float(W + 2),
                                Alu.max, Alu.min)

        mwy0 = f()
        nc.vector.tensor_tensor(mwy0[:], mask[:], wy0[:], Alu.mult)
        mwy1 = f()
        nc.vector.tensor_tensor(mwy1[:], mask[:], wy1[:], Alu.mult)
        c00 = f()
        nc.vector.tensor_tensor(c00[:], mwy0[:], wx0[:], Alu.mult)
        c01 = f()
        nc.vector.tensor_tensor(c01[:], mwy0[:], wx1[:], Alu.mult)
        c10 = f()
        nc.vector.tensor_tensor(c10[:], mwy1[:], wx0[:], Alu.mult)
        c11 = f()
        nc.vector.tensor_tensor(c11[:], mwy1[:], wx1[:], Alu.mult)

        # row index r = (y0+PADG)*GW + (x0+PADG), written k-major: gKM[p][k][s]
        gAf = f()
        nc.vector.tensor_scalar(gAf[:], y0f[:], float(GW), float(PADG * GW + PADG),
                                Alu.mult, Alu.add)
        gKM = dpool.tile([128, K2, NS], F32, tag="gKM", name="gKM")
        gKM_w = AP(gKM[:].tensor, gKM[:].offset, [[K2 * NS, 128], [1, NS], [NS, K2]])
        nc.vector.tensor_tensor(gKM_w, gAf[:], x0f[:], Alu.add)

        # ---------- idx wrap via PE transposes ----------
        # want idxAw[16u'+pp][k*256 + s*8 + u] = gKM[16u+pp][k][s] for all u'
        idxAw = spool.tile([128, K2 * 256], I16)
        t1s = []
        for g in range(3):  # pass 1: [128, 96] -> [96, 128], 96 = 3 taps x 32 s
            trp = opool.tile([128, 512], F32, tag="om", name="omp")
            nc.tensor.transpose(
                trp[0:96, 0:128],
                gKM[:].rearrange("p a b -> p (a b)")[:, 96 * g:96 * (g + 1)],
                ident[:],
            )
            t1 = spool.tile([96, 128], F32, tag=f"t1_{g}")
            nc.scalar.copy(t1[:], trp[0:96, 0:128])
            t1s.append(t1)
        for g in range(3):
            for u in range(8):  # pass 2: [96, 16] -> [16, 96]
                trp = opool.tile([128, 512], F32, tag="om", name="omp")
                nc.tensor.transpose(
                    trp[0:16, 0:96], t1s[g][:, 16 * u:16 * u + 16],
                    ident[0:96, 0:96],
                )
                t2 = vpool.tile([16, 96], F32, tag="t2", name="t2")
                nc.scalar.copy(t2[:], trp[0:16, 0:96])
                # scatter (k', s) -> col (3g+k')*256 + s*8 + u, cast f32->i16
                dst = AP(
                    idxAw[:].tensor,
                    idxAw[:].offset + (3 * g) * 256 + u,
                    [[K2 * 256, 16], [256, 3], [8, NS]],
                )
                nc.vector.tensor_copy(
                    dst, t2[:].rearrange("p (a b) -> p a b", a=3)
                )
        for u in range(1, 8):  # replicate to all 8 16-partition groups
            nc.sync.dma_start(idxAw[16 * u:16 * u + 16, :], idxAw[0:16, :])

        # ---------- main loop ----------
        out_sb = spool.tile([128, HW], F32)
        xt2_src = AP(xt2_d.tensor, 0, [[256, GROWS - 1], [1, 512]])
        for h in range(NHALF):
            outp = ppool.tile([128, PPH], F32, tag="out", name="outp")
            for k in range(K2):
                gb = gpool.tile([128, SPH, 512], BF16, tag="gb", name="gb")
                nc.gpsimd.dma_gather(
                    gb[:], xt2_src,
                    idxAw[:, k * 256 + 128 * h: k * 256 + 128 * h + 128],
                    PPH, PPH, 512, elem_step=256, single_packet=False,
                )
                trp = None
                for t in range(SPH):
                    s = h * SPH + t
                    # corners: [0:128]=A0(c00) [128:256]=B0(c10)
                    #          [256:384]=A1(c01) [384:512]=B1(c11)
                    m = vpool.tile([128, 128], BF16, tag="m", name="m")
                    nc.scalar.activation(m[:], gb[:, t, 0:128], IDENT,
                                         bias=0.0, scale=c00[:, s, k:k + 1])
                    v = vpool.tile([128, 128], BF16, tag="v", name="v")
                    nc.vector.scalar_tensor_tensor(
                        v[:], gb[:, t, 256:384], c01[:, s, k:k + 1], m[:],
                        Alu.mult, Alu.add)
                    nc.vector.scalar_tensor_tensor(
                        v[:], gb[:, t, 128:256], c10[:, s, k:k + 1], v[:],
                        Alu.mult, Alu.add)
                    nc.vector.scalar_tensor_tensor(
                        v[:], gb[:, t, 384:512], c11[:, s, k:k + 1], v[:],
                        Alu.mult, Alu.add)
                    if t % 4 == 0:
                        trp = tpool.tile([128, 512], BF16, tag="tr", name="trp")
                    nc.tensor.transpose(trp[:, (t % 4) * 128:(t % 4) * 128 + 128],
                                        v[:], identb[:])
                    if t % 4 == 3:
                        vT = vpool.tile([128, 512], BF16, tag="vT", name="vT")
                        nc.scalar.copy(vT[:], trp[:])
                        bk = t // 4
                        nc.tensor.matmul(
                            outp[:, bk * 512:(bk + 1) * 512], wT[:, k, :], vT[:],
                            start=(k == 0), stop=(k == K2 - 1),
                        )
            for bk in range(4):
                nc.scalar.activation(
                    out_sb[:, h * PPH + bk * 512: h * PPH + (bk + 1) * 512],
                    outp[:, bk * 512:(bk + 1) * 512],
                    IDENT, bias=bias_sb[:], scale=1.0,
                )
        nc.sync.dma_start(out_d[:, :], out_sb[:])


def _make_consts():
    c = np.zeros((128, 707), np.float32)
    c[:, 0:128] = np.eye(128, dtype=np.float32)
    p = np.arange(128)
    c[:, 128] = p
    c[:, 129] = (p >= 64)
    c[:, 130] = p % 64
    s = np.arange(32)[:, None, None]
    kyv = np.arange(3)[None, :, None]
    kxv = np.arange(3)[None, None, :]
    c[:, 131:419] = np.broadcast_to(
        (2 * s + kyv - 1 + 0 * kxv).reshape(-1), (128, 288))
    c[:, 419:707] = np.broadcast_to(
        (0 * s + 0 * kyv + kxv - 1).reshape(-1), (128, 288))
    return c


_COMPILED = None


def _get_compiled():
    global _COMPILED
    if _COMPILED is None:
        nc = bacc.Bacc(get_trn_type() or "TRN2", target_bir_lowering=False,
                       debug=False, num_devices=B)
        with tile.TileContext(nc) as tc:
            _emit(tc)
        nc.compile()
        _COMPILED = nc
    return _COMPILED


def kernel(x, w_om, b_om, weight, bias):
    global LAST_EXEC_TIME_NS, LAST_RESULT
    x = np.ascontiguousarray(np.asarray(x, dtype=np.float32))
    w_om_f = np.ascontiguousarray(np.asarray(w_om, np.float32).reshape(27, 1152))
    b_om_f = np.ascontiguousarray(np.asarray(b_om, np.float32).reshape(27, 1))
    weight_f = np.ascontiguousarray(np.asarray(weight, np.float32).reshape(128, 1152))
    bias_f = np.ascontiguousarray(np.asarray(bias, np.float32).reshape(128, 1))

    nc = _get_compiled()
    consts = _make_consts()
    in_maps = [
        {
            "x": np.ascontiguousarray(x[b].reshape(C, HW)),
            "w_om": w_om_f,
            "b_om": b_om_f,
            "weight": weight_f,
            "bias": bias_f,
            "consts": consts,
        }
        for b in range(B)
    ]
    trace = bool(os.environ.get("DCN_TRACE"))
    res = run_bass_kernel_spmd(nc, in_maps, core_ids=list(range(B)), trace=trace)
    LAST_RESULT = res
    LAST_EXEC_TIME_NS = res.exec_time_ns
    out = np.stack([res.results[b]["out"].reshape(C, H, W) for b in range(B)])
    return out.astype(np.float32)


# revision 7
# speedup vs baseline: 2.4585x; 1.2947x over previous
"""DCNv2 (modulated deformable convolution) on 8 Trainium2 NeuronCores.

kernel(**inputs) takes the full unsharded inputs
    x      (8, 128, 64, 64) f32
    w_om   (27, 128, 3, 3)  f32
    b_om   (27,)            f32
    weight (128, 128, 3, 3) f32
    bias   (128,)           f32
and returns the full output (8, 128, 64, 64) f32.

Sharding: pure data-parallel over batch - one image per NeuronCore, small
weights replicated; no collectives.

v3 per-core program (bf16 datapath):
  1. x/weights are cast to bf16 during the load DMA (SWDGE); x is staged
     twice into a DRAM image xt2[GROWS, 256] where row r=(y,x) holds
     [C(y,x), C(y+1,x)] - the 4 bilinear corners of any sample are 4*128
     CONTIGUOUS bf16 values (one 1KB gather descriptor per (pixel, tap)).
     Staging writes are split across the sync and scalar HWDGE rings so
     they don't serialize behind each other.
  2. offset conv (27ch 3x3) on the PE in bf16; sampling positions + gather
     row indices are computed FIRST (the softmax mask / bilinear
     coefficients follow and overlap the first gathers); indices are
     wrapped into the 16-partition dma_gather layout with PE transposes.
  3. per (half, tap) one dma_gather fetches [A0 B0 A1 B1] corner blocks in
     (pixel-partition, channel) layout; corners are combined at
     whole-gather granularity: ACT applies c00 per pixel-tile (16 ops into
     one [128,16,128] tile), DVE does 3 broadcast-coefficient multiplies +
     3 bf16 adds; the result is PE-transposed back to (channel, pixel)
     (4 tiles per PSUM bank) and accumulated over the 9 taps into PSUM
     with the 128x128x3x3 weight; bias is added on the PSUM->SBUF copy.
"""

import os
import sys

import numpy as np

sys.path.insert(0, "/opt/trn_rl_repo")

from contextlib import ExitStack

import concourse.bacc as bacc
import concourse.mybir as mybir
import concourse.tile as tile
from concourse._compat import get_trn_type
from concourse.alu_op_type import AluOpType as Alu
from concourse.bass import AP
from concourse.bass_utils import run_bass_kernel_spmd
from concourse import library_config

F32 = mybir.dt.float32
BF16 = mybir.dt.bfloat16
I32 = mybir.dt.int32
I16 = mybir.dt.int16

B = 8
C = 128
H = W = 64
HW = H * W
K2 = 9
PADG = 4
GW = H + 2 * PADG      # 72
GROWS = GW * GW        # 5184
NS = 32
NHALF = 2
SPH = NS // NHALF      # 16 s-tiles per half
PPH = HW // NHALF      # 2048 pixels per half
IDENT = mybir.ActivationFunctionType.Identity

LAST_EXEC_TIME_NS = None
LAST_RESULT = None


def _emit(tc):
    nc = tc.nc
    x_d = nc.dram_tensor("x", [C, HW], F32, kind="ExternalInput").ap()
    w_om_d = nc.dram_tensor("w_om", [27, 1152], F32, kind="ExternalInput").ap()
    b_om_d = nc.dram_tensor("b_om", [27, 1], F32, kind="ExternalInput").ap()
    weight_d = nc.dram_tensor("weight", [C, 1152], F32, kind="ExternalInput").ap()
    bias_d = nc.dram_tensor("bias", [C, 1], F32, kind="ExternalInput").ap()
    out_d = nc.dram_tensor("out", [C, HW], F32, kind="ExternalOutput").ap()
    xt2_d = nc.dram_tensor("xt2_pad", [GROWS, 256], BF16, kind="Internal").ap()
    consts_d = nc.dram_tensor("consts", [128, 707], F32, kind="ExternalInput").ap()

    nc.gpsimd.load_library(library_config.mlp)

    ctx = ExitStack()
    with ctx:
        cpool = ctx.enter_context(tc.tile_pool(name="const", bufs=1))
        spool = ctx.enter_context(tc.tile_pool(name="setup", bufs=1))
        dpool = ctx.enter_context(tc.tile_pool(name="data", bufs=1))
        gpool = ctx.enter_context(tc.tile_pool(name="gath", bufs=3))
        vpool = ctx.enter_context(tc.tile_pool(name="val", bufs=2))
        ppool = ctx.enter_context(tc.tile_pool(name="psum", bufs=1, space="PSUM"))
        tpool = ctx.enter_context(tc.tile_pool(name="trps", bufs=2, space="PSUM"))
        opool = ctx.enter_context(tc.tile_pool(name="omps", bufs=2, space="PSUM"))

        # ---------- loads (bf16 casts during DMA on the SWDGE ring; small
        # f32 tensors on the sync HWDGE ring, queued before staging) ----------
        cons = cpool.tile([128, 707], F32)
        nc.sync.dma_start(cons[:], consts_d[:, :])
        ident = cons[:, 0:128]
        hob = cons[:, 129:130]
        wo_r = cons[:, 130:131]
        ykc = cons[:, 131:419]
        xkc = cons[:, 419:707]

        x16 = spool.tile([128, HW], BF16)
        nc.gpsimd.dma_start(x16[:], x_d[:, :])
        w16 = spool.tile([128, 1152], BF16)
        nc.gpsimd.dma_start(w16[:], weight_d[:, :])
        w_om16 = spool.tile([27, 1152], BF16)
        nc.gpsimd.dma_start(w_om16[:], w_om_d[:, :])

        b_om_sb = spool.tile([27, 1], F32)
        nc.sync.dma_start(b_om_sb[:], b_om_d[:, :])
        bias_sb = spool.tile([128, 1], F32)
        nc.sync.dma_start(bias_sb[:], bias_d[:, :])

        identb = spool.tile([128, 128], BF16)
        nc.vector.tensor_copy(identb[:], ident)

        # ---------- x_pad (bf16) for the offset conv ----------
        XP = 66
        x_pad = spool.tile([128, XP * XP], BF16)
        nc.vector.memset(x_pad[:], 0.0)
        nc.vector.tensor_copy(
            x_pad[:].rearrange("p (a b) -> p a b", a=XP)[:, 1:65, 1:65],
            x16[:].rearrange("p (a b) -> p a b", a=64),
        )

        # ---------- weight transposes ----------
        womT = spool.tile([128, K2, 27], BF16)
        for k in range(K2):
            trp = tpool.tile([128, 512], BF16, tag="tr", name="trp")
            nc.tensor.transpose(
                trp[:, 0:27],
                w_om16[:].rearrange("p (c k) -> p c k", k=K2)[:, :, k],
                identb[0:27, 0:27],
            )
            nc.scalar.copy(womT[:, k, :], trp[:, 0:27])
        wT = spool.tile([128, K2, 128], BF16)
        for k in range(K2):
            trp = tpool.tile([128, 512], BF16, tag="tr", name="trp")
            nc.tensor.transpose(
                trp[:, 0:128],
                w16[:].rearrange("p (c k) -> p c k", k=K2)[:, :, k], identb[:],
            )
            nc.scalar.copy(wT[:, k, :], trp[:, 0:128])

        # ---------- offset conv: om (27, 4096) f32 ----------
        om_sb = spool.tile([27, HW], F32)
        xpv = x_pad[:].rearrange("p (a b) -> p a b", a=XP)
        for ch in range(8):
            omp = opool.tile([128, 512], F32, tag="om", name="omp")
            for k in range(K2):
                dy_, dx_ = k // 3, k % 3
                r0 = ch * 8 + dy_
                nc.tensor.matmul(
                    omp[0:27, :], womT[:, k, :], xpv[:, r0:r0 + 8, dx_:dx_ + 64],
                    start=(k == 0), stop=(k == K2 - 1),
                )
            nc.scalar.activation(
                om_sb[:, ch * 512:(ch + 1) * 512], omp[0:27, :],
                IDENT, bias=b_om_sb[:], scale=1.0,
            )

        # ---------- omT (128 pix, 27) per s-tile; 4 tiles per psum copy ----
        omT = spool.tile([128, NS, 27], F32)
        for s4 in range(NS // 4):
            trp = opool.tile([128, 512], F32, tag="om", name="omp")
            for j in range(4):
                nc.tensor.transpose(
                    trp[:, j * 128:j * 128 + 27],
                    om_sb[:, (4 * s4 + j) * 128:(4 * s4 + j + 1) * 128],
                    ident[0:27, 0:27],
                )
            nc.scalar.copy(
                omT[:, 4 * s4:4 * s4 + 4, :],
                trp[:].rearrange("p (a b) -> p a b", b=128)[:, :, 0:27],
            )

        # ---------- zero-fill xt2 on the scalar HWDGE ring ----------
        zt = spool.tile([128, 1296], BF16)
        nc.vector.memset(zt[:], 0.0)
        for i in range(8):
            nc.scalar.dma_start(
                AP(xt2_d.tensor, i * 128 * 1296, [[1296, 128], [1, 1296]]), zt[:]
            )

        # ---------- stage xt2: transpose x16 and write row-pairs ----------
        # tile s holds pixels of image rows y=2s,2s+1 (64 x each) on partitions
        for s in range(NS):
            trp = tpool.tile([128, 512], BF16, tag="tr", name="trp")
            nc.tensor.transpose(trp[:, 0:128], x16[:, s * 128:(s + 1) * 128],
                                identb[:])
            stg = vpool.tile([128, 128], BF16, tag="stg", name="stg")
            nc.scalar.copy(stg[:], trp[:, 0:128])
            # first halves: xt2[(2s+ry+PADG)*GW + PADG + x][0:128] = C(2s+ry, x)
            nc.sync.dma_start(
                AP(xt2_d.tensor, ((2 * s + PADG) * GW + PADG) * 256,
                   [[GW * 256, 2], [256, 64], [1, 128]]),
                stg[:],
            )
            # second halves: xt2[(2s+ry-1+PADG)*GW + PADG + x][128:256] = C(2s+ry, x)
            nc.scalar.dma_start(
                AP(xt2_d.tensor, ((2 * s - 1 + PADG) * GW + PADG) * 256 + 128,
                   [[GW * 256, 2], [256, 64], [1, 128]]),
                stg[:],
            )

        # ---------- sampling positions -> gather indices (before mask) ----
        _cnt = [0]

        def f(shape=(128, NS, K2), dt=F32, tag=None):
            _cnt[0] += 1
            nm = f"cf{_cnt[0]}"
            return dpool.tile(list(shape), dt, tag=tag or nm, name=nm)

        omT_t = omT[:].tensor
        omT_off = omT[:].offset
        dyT = AP(omT_t, omT_off + 0, [[NS * 27, 128], [27, NS], [2, K2]])
        dxT = AP(omT_t, omT_off + 1, [[NS * 27, 128], [27, NS], [2, K2]])
        mlg = omT[:, :, 18:27]

        ykv = ykc.rearrange("p (s a) -> p s a", a=K2)
        xkv = xkc.rearrange("p (s a) -> p s a", a=K2)
        py = f()
        nc.vector.scalar_tensor_tensor(py[:], dyT, hob, ykv, Alu.add, Alu.add)
        px = f()
        nc.vector.scalar_tensor_tensor(px[:], dxT, wo_r, xkv, Alu.add, Alu.add)

        def floorit(v):
            vi = f(dt=I32, tag="fl_i")
            nc.vector.tensor_copy(vi[:], v[:])
            v0 = f(tag="fl_f")
            nc.vector.tensor_copy(v0[:], vi[:])
            gt = f(tag="fl_gt")
            nc.vector.tensor_tensor(gt[:], v0[:], v[:], Alu.is_gt)
            v0f = f()
            nc.vector.tensor_tensor(v0f[:], v0[:], gt[:], Alu.subtract)
            return v0f

        y0f = floorit(py)
        x0f = floorit(px)
        nc.vector.tensor_scalar(y0f[:], y0f[:], -float(PADG), float(H + 2),
                                Alu.max, Alu.min)
        nc.vector.tensor_scalar(x0f[:], x0f[:], -float(PADG), float(W + 2),
                                Alu.max, Alu.min)

        # row index r = (y0+PADG)*GW + (x0+PADG), written k-major: gKM[p][k][s]
        gAf = f()
        nc.vector.tensor_scalar(gAf[:], y0f[:], float(GW), float(PADG * GW + PADG),
                                Alu.mult, Alu.add)
        gKM = dpool.tile([128, K2, NS], F32, tag="gKM", name="gKM")
        gKM_w = AP(gKM[:].tensor, gKM[:].offset, [[K2 * NS, 128], [1, NS], [NS, K2]])
        nc.vector.tensor_tensor(gKM_w, gAf[:], x0f[:], Alu.add)

        # idx wrap via PE transposes:
        # idxAw[16u'+pp][k*256 + s*8 + u] = gKM[16u+pp][k][s] for all u'
        idxAw = spool.tile([128, K2 * 256], I16)
        t1s = []
        for g in range(3):  # pass 1: [128, 96] -> [96, 128], 96 = 3 taps x 32 s
            trp = opool.tile([128, 512], F32, tag="om", name="omp")
            nc.tensor.transpose(
                trp[0:96, 0:128],
                gKM[:].rearrange("p a b -> p (a b)")[:, 96 * g:96 * (g + 1)],
                ident[:],
            )
            t1 = spool.tile([96, 128], F32, tag=f"t1_{g}")
            nc.scalar.copy(t1[:], trp[0:96, 0:128])
            t1s.append(t1)
        for g in range(3):
            for u4 in range(2):  # pass 2: 4x [96, 16] -> [16, 96] per psum buf
                trp = opool.tile([128, 512], F32, tag="om", name="omp")
                for j in range(4):
                    u = 4 * u4 + j
                    nc.tensor.transpose(
                        trp[0:16, j * 128:j * 128 + 96],
                        t1s[g][:, 16 * u:16 * u + 16],
                        ident[0:96, 0:96],
                    )
                t2 = vpool.tile([16, 512], F32, tag="t2", name="t2")
                nc.scalar.copy(t2[:], trp[0:16, :])
                # scatter (j, k', s) -> col (3g+k')*256 + s*8 + (4*u4+j)
                dst = AP(
                    idxAw[:].tensor,
                    idxAw[:].offset + (3 * g) * 256 + 4 * u4,
                    [[K2 * 256, 16], [1, 4], [256, 3], [8, NS]],
                )
                src = AP(
                    t2[:].tensor, t2[:].offset,
                    [[512, 16], [128, 4], [32, 3], [1, 32]],
                )
                nc.vector.tensor_copy(dst, src)
        for u in range(1, 8):  # replicate to all 8 16-partition groups
            nc.sync.dma_start(idxAw[16 * u:16 * u + 16, :], idxAw[0:16, :])

        # ---------- softmax mask + bilinear coefficients (overlap gathers) --
        e = f()
        nc.scalar.activation(e[:], mlg, mybir.ActivationFunctionType.Exp)
        ssum = f((128, NS, 1))
        nc.vector.tensor_reduce(ssum[:], e[:], mybir.AxisListType.X, Alu.add)
        rs = f((128, NS, 1))
        nc.vector.reciprocal(rs[:], ssum[:])
        mask = f()
        nc.vector.tensor_tensor(mask[:], e[:], rs[:].to_broadcast([128, NS, K2]),
                                Alu.mult)

        wy1 = f()
        nc.vector.tensor_tensor(wy1[:], py[:], y0f[:], Alu.subtract)
        wy0 = f()
        nc.vector.tensor_scalar(wy0[:], wy1[:], -1.0, 1.0, Alu.mult, Alu.add)
        wx1 = f()
        nc.vector.tensor_tensor(wx1[:], px[:], x0f[:], Alu.subtract)
        wx0 = f()
        nc.vector.tensor_scalar(wx0[:], wx1[:], -1.0, 1.0, Alu.mult, Alu.add)

        mwy0 = f()
        nc.vector.tensor_tensor(mwy0[:], mask[:], wy0[:], Alu.mult)
        mwy1 = f()
        nc.vector.tensor_tensor(mwy1[:], mask[:], wy1[:], Alu.mult)
        c00 = f()
        nc.vector.tensor_tensor(c00[:], mwy0[:], wx0[:], Alu.mult)
        c01 = f()
        nc.vector.tensor_tensor(c01[:], mwy0[:], wx1[:], Alu.mult)
        c10 = f()
        nc.vector.tensor_tensor(c10[:], mwy1[:], wx0[:], Alu.mult)
        c11 = f()
        nc.vector.tensor_tensor(c11[:], mwy1[:], wx1[:], Alu.mult)

        # wy1-py etc. consume py/px AFTER idx path; mask ops overlap gathers

        # ---------- main loop ----------
        out_sb = spool.tile([128, HW], F32)
        xt2_src = AP(xt2_d.tensor, 0, [[256, GROWS - 1], [1, 512]])
        for h in range(NHALF):
            outp = ppool.tile([128, PPH], F32, tag="out", name="outp")
            for k in range(K2):
                gb = gpool.tile([128, SPH, 512], BF16, tag="gb", name="gb")
                nc.gpsimd.dma_gather(
                    gb[:], xt2_src,
                    idxAw[:, k * 256 + 128 * h: k * 256 + 128 * h + 128],
                    PPH, PPH, 512, elem_step=256, single_packet=False,
                )
                # corners: [0:128]=A0(c00) [128:256]=B0(c10)
                #          [256:384]=A1(c01) [384:512]=B1(c11)
                mb = vpool.tile([128, SPH, 128], BF16, tag="mb", name="mb")
                for t in range(SPH):
                    s = h * SPH + t
                    nc.scalar.activation(mb[:, t, :], gb[:, t, 0:128], IDENT,
                                         bias=0.0, scale=c00[:, s, k:k + 1])
                hs = h * SPH
                u1 = vpool.tile([128, SPH, 128], BF16, tag="u1", name="u1")
                nc.vector.tensor_tensor(
                    u1[:], gb[:, :, 256:384],
                    c01[:, hs:hs + SPH, k:k + 1].to_broadcast([128, SPH, 128]),
                    Alu.mult)
                u2 = vpool.tile([128, SPH, 128], BF16, tag="u2", name="u2")
                nc.vector.tensor_tensor(
                    u2[:], gb[:, :, 128:256],
                    c10[:, hs:hs + SPH, k:k + 1].to_broadcast([128, SPH, 128]),
                    Alu.mult)
                u3 = vpool.tile([128, SPH, 128], BF16, tag="u3", name="u3")
                nc.vector.tensor_tensor(
                    u3[:], gb[:, :, 384:512],
                    c11[:, hs:hs + SPH, k:k + 1].to_broadcast([128, SPH, 128]),
                    Alu.mult)
                vb = vpool.tile([128, SPH, 128], BF16, tag="vb", name="vb")
                nc.vector.tensor_tensor(vb[:], u1[:], mb[:], Alu.add)
                nc.vector.tensor_tensor(vb[:], vb[:], u2[:], Alu.add)
                nc.vector.tensor_tensor(vb[:], vb[:], u3[:], Alu.add)

                trp = None
                for t in range(SPH):
                    if t % 4 == 0:
                        trp = tpool.tile([128, 512], BF16, tag="tr", name="trp")
                    nc.tensor.transpose(trp[:, (t % 4) * 128:(t % 4) * 128 + 128],
                                        vb[:, t, :], identb[:])
                    if t % 4 == 3:
                        vT = vpool.tile([128, 512], BF16, tag="vT", name="vT")
                        nc.scalar.copy(vT[:], trp[:])
                        bk = t // 4
                        nc.tensor.matmul(
                            outp[:, bk * 512:(bk + 1) * 512], wT[:, k, :], vT[:],
                            start=(k == 0), stop=(k == K2 - 1),
                        )
            for bk in range(4):
                nc.scalar.activation(
                    out_sb[:, h * PPH + bk * 512: h * PPH + (bk + 1) * 512],
                    outp[:, bk * 512:(bk + 1) * 512],
                    IDENT, bias=bias_sb[:], scale=1.0,
                )
        nc.sync.dma_start(out_d[:, :], out_sb[:])


def _make_consts():
    c = np.zeros((128, 707), np.float32)
    c[:, 0:128] = np.eye(128, dtype=np.float32)
    p = np.arange(128)
    c[:, 128] = p
    c[:, 129] = (p >= 64)
    c[:, 130] = p % 64
    s = np.arange(32)[:, None, None]
    kyv = np.arange(3)[None, :, None]
    kxv = np.arange(3)[None, None, :]
    c[:, 131:419] = np.broadcast_to(
        (2 * s + kyv - 1 + 0 * kxv).reshape(-1), (128, 288))
    c[:, 419:707] = np.broadcast_to(
        (0 * s + 0 * kyv + kxv - 1).reshape(-1), (128, 288))
    return c


_COMPILED = None


def _get_compiled():
    global _COMPILED
    if _COMPILED is None:
        nc = bacc.Bacc(get_trn_type() or "TRN2", target_bir_lowering=False,
                       debug=False, num_devices=B)
        with tile.TileContext(nc) as tc:
            _emit(tc)
        nc.compile()
        _COMPILED = nc
    return _COMPILED


def kernel(x, w_om, b_om, weight, bias):
    global LAST_EXEC_TIME_NS, LAST_RESULT
    x = np.ascontiguousarray(np.asarray(x, dtype=np.float32))
    w_om_f = np.ascontiguousarray(np.asarray(w_om, np.float32).reshape(27, 1152))
    b_om_f = np.ascontiguousarray(np.asarray(b_om, np.float32).reshape(27, 1))
    weight_f = np.ascontiguousarray(np.asarray(weight, np.float32).reshape(128, 1152))
    bias_f = np.ascontiguousarray(np.asarray(bias, np.float32).reshape(128, 1))

    nc = _get_compiled()
    consts = _make_consts()
    in_maps = [
        {
            "x": np.ascontiguousarray(x[b].reshape(C, HW)),
            "w_om": w_om_f,
            "b_om": b_om_f,
            "weight": weight_f,
            "bias": bias_f,
            "consts": consts,
        }
        for b in range(B)
    ]
    trace = bool(os.environ.get("DCN_TRACE"))
    res = run_bass_kernel_spmd(nc, in_maps, core_ids=list(range(B)), trace=trace)
    LAST_RESULT = res
    LAST_EXEC_TIME_NS = res.exec_time_ns
    out = np.stack([res.results[b]["out"].reshape(C, H, W) for b in range(B)])
    return out.astype(np.float32)


# revision 8
# speedup vs baseline: 2.5873x; 1.0524x over previous
"""DCNv2 (modulated deformable convolution) on 8 Trainium2 NeuronCores.

kernel(**inputs) takes the full unsharded inputs
    x      (8, 128, 64, 64) f32
    w_om   (27, 128, 3, 3)  f32
    b_om   (27,)            f32
    weight (128, 128, 3, 3) f32
    bias   (128,)           f32
and returns the full output (8, 128, 64, 64) f32.

Sharding: pure data-parallel over batch - one image per NeuronCore, small
weights replicated; no collectives.

v3 per-core program (bf16 datapath):
  1. x/weights are cast to bf16 during the load DMA (SWDGE); x is staged
     twice into a DRAM image xt2[GROWS, 256] where row r=(y,x) holds
     [C(y,x), C(y+1,x)] - the 4 bilinear corners of any sample are 4*128
     CONTIGUOUS bf16 values (one 1KB gather descriptor per (pixel, tap)).
     Staging writes are split across the sync and scalar HWDGE rings so
     they don't serialize behind each other.
  2. offset conv (27ch 3x3) on the PE in bf16; sampling positions + gather
     row indices are computed FIRST (the softmax mask / bilinear
     coefficients follow and overlap the first gathers); indices are
     wrapped into the 16-partition dma_gather layout with PE transposes.
  3. per (half, tap) one dma_gather fetches [A0 B0 A1 B1] corner blocks in
     (pixel-partition, channel) layout; corners are combined at
     whole-gather granularity: ACT applies c00 per pixel-tile (16 ops into
     one [128,16,128] tile), DVE does 3 broadcast-coefficient multiplies +
     3 bf16 adds; the result is PE-transposed back to (channel, pixel)
     (4 tiles per PSUM bank) and accumulated over the 9 taps into PSUM
     with the 128x128x3x3 weight; bias is added on the PSUM->SBUF copy.
"""

import os
import sys

import numpy as np

sys.path.insert(0, "/opt/trn_rl_repo")

from contextlib import ExitStack

import concourse.bacc as bacc
import concourse.mybir as mybir
import concourse.tile as tile
from concourse._compat import get_trn_type
from concourse.alu_op_type import AluOpType as Alu
from concourse.bass import AP
from concourse.bass_utils import run_bass_kernel_spmd
from concourse import library_config

F32 = mybir.dt.float32
BF16 = mybir.dt.bfloat16
I32 = mybir.dt.int32
I16 = mybir.dt.int16

B = 8
C = 128
H = W = 64
HW = H * W
K2 = 9
PADG = 4
GW = H + 2 * PADG      # 72
GROWS = GW * GW        # 5184
NS = 32
NHALF = 2
SPH = NS // NHALF      # 16 s-tiles per half
PPH = HW // NHALF      # 2048 pixels per half
IDENT = mybir.ActivationFunctionType.Identity

LAST_EXEC_TIME_NS = None
LAST_RESULT = None
SINGLE_PACKET = bool(int(os.environ.get("DCN_SP", "0")))


def _emit(tc):
    nc = tc.nc
    x_d = nc.dram_tensor("x", [C, HW], F32, kind="ExternalInput").ap()
    w_om_d = nc.dram_tensor("w_om", [27, 1152], F32, kind="ExternalInput").ap()
    b_om_d = nc.dram_tensor("b_om", [27, 1], F32, kind="ExternalInput").ap()
    weight_d = nc.dram_tensor("weight", [C, 1152], F32, kind="ExternalInput").ap()
    bias_d = nc.dram_tensor("bias", [C, 1], F32, kind="ExternalInput").ap()
    out_d = nc.dram_tensor("out", [C, HW], F32, kind="ExternalOutput").ap()
    xt2_d = nc.dram_tensor("xt2_pad", [GROWS, 256], BF16, kind="Internal").ap()
    consts_d = nc.dram_tensor("consts", [128, 707], F32, kind="ExternalInput").ap()

    ctx = ExitStack()
    with ctx:
        cpool = ctx.enter_context(tc.tile_pool(name="const", bufs=1))
        spool = ctx.enter_context(tc.tile_pool(name="setup", bufs=1))
        dpool = ctx.enter_context(tc.tile_pool(name="data", bufs=1))
        gpool = ctx.enter_context(tc.tile_pool(name="gath", bufs=3))
        vpool = ctx.enter_context(tc.tile_pool(name="val", bufs=2))
        ppool = ctx.enter_context(tc.tile_pool(name="psum", bufs=1, space="PSUM"))
        tpool = ctx.enter_context(tc.tile_pool(name="trps", bufs=2, space="PSUM"))
        opool = ctx.enter_context(tc.tile_pool(name="omps", bufs=2, space="PSUM"))

        # ---------- loads (bf16 casts during DMA on the SWDGE ring; small
        # f32 tensors on the sync HWDGE ring, queued before staging) ----------
        cons = cpool.tile([128, 707], F32)
        nc.sync.dma_start(cons[:], consts_d[:, :])
        ident = cons[:, 0:128]
        hob = cons[:, 129:130]
        wo_r = cons[:, 130:131]
        ykc = cons[:, 131:419]
        xkc = cons[:, 419:707]

        x16 = spool.tile([128, HW], BF16)
        nc.gpsimd.dma_start(x16[:], x_d[:, :])
        w16 = spool.tile([128, 1152], BF16)
        nc.gpsimd.dma_start(w16[:], weight_d[:, :])
        w_om16 = spool.tile([27, 1152], BF16)
        nc.gpsimd.dma_start(w_om16[:], w_om_d[:, :])
        nc.gpsimd.load_library(library_config.mlp)

        b_om_sb = spool.tile([27, 1], F32)
        nc.sync.dma_start(b_om_sb[:], b_om_d[:, :])
        bias_sb = spool.tile([128, 1], F32)
        nc.sync.dma_start(bias_sb[:], bias_d[:, :])

        identb = spool.tile([128, 128], BF16)
        nc.vector.tensor_copy(identb[:], ident)

        # ---------- x_pad (bf16) for the offset conv ----------
        XP = 66
        x_pad = spool.tile([128, XP * XP], BF16)
        nc.vector.memset(x_pad[:], 0.0)
        nc.vector.tensor_copy(
            x_pad[:].rearrange("p (a b) -> p a b", a=XP)[:, 1:65, 1:65],
            x16[:].rearrange("p (a b) -> p a b", a=64),
        )

        # ---------- weight transposes ----------
        womT = spool.tile([128, K2, 27], BF16)
        for k in range(K2):
            trp = tpool.tile([128, 512], BF16, tag="tr", name="trp")
            nc.tensor.transpose(
                trp[:, 0:27],
                w_om16[:].rearrange("p (c k) -> p c k", k=K2)[:, :, k],
                identb[0:27, 0:27],
            )
            nc.scalar.copy(womT[:, k, :], trp[:, 0:27])
        wT = spool.tile([128, K2, 128], BF16)
        for k in range(K2):
            trp = tpool.tile([128, 512], BF16, tag="tr", name="trp")
            nc.tensor.transpose(
                trp[:, 0:128],
                w16[:].rearrange("p (c k) -> p c k", k=K2)[:, :, k], identb[:],
            )
            nc.scalar.copy(wT[:, k, :], trp[:, 0:128])

        # ---------- offset conv: om (27, 4096) f32 ----------
        om_sb = spool.tile([27, HW], F32)
        xpv = x_pad[:].rearrange("p (a b) -> p a b", a=XP)
        for ch in range(8):
            omp = opool.tile([128, 512], F32, tag="om", name="omp")
            for k in range(K2):
                dy_, dx_ = k // 3, k % 3
                r0 = ch * 8 + dy_
                nc.tensor.matmul(
                    omp[0:27, :], womT[:, k, :], xpv[:, r0:r0 + 8, dx_:dx_ + 64],
                    start=(k == 0), stop=(k == K2 - 1),
                )
            nc.scalar.activation(
                om_sb[:, ch * 512:(ch + 1) * 512], omp[0:27, :],
                IDENT, bias=b_om_sb[:], scale=1.0,
            )

        # ---------- omT (128 pix, 27) per s-tile; 4 tiles per psum copy ----
        omT = spool.tile([128, NS, 27], F32)
        for s4 in range(NS // 4):
            trp = opool.tile([128, 512], F32, tag="om", name="omp")
            for j in range(4):
                nc.tensor.transpose(
                    trp[:, j * 128:j * 128 + 27],
                    om_sb[:, (4 * s4 + j) * 128:(4 * s4 + j + 1) * 128],
                    ident[0:27, 0:27],
                )
            nc.scalar.copy(
                omT[:, 4 * s4:4 * s4 + 4, :],
                trp[:].rearrange("p (a b) -> p a b", b=128)[:, :, 0:27],
            )

        # ---------- zero-fill xt2 on the scalar HWDGE ring ----------
        zt = spool.tile([128, 1296], BF16)
        nc.vector.memset(zt[:], 0.0)
        for i in range(8):
            nc.scalar.dma_start(
                AP(xt2_d.tensor, i * 128 * 1296, [[1296, 128], [1, 1296]]), zt[:]
            )

        # ---------- stage xt2 ----------
        # xt2 row (y+PADG, x) = [C(y,x), C(y+1,x)].  Per s we build the two
        # full rows y=2s,2s+1 as a [64, 512] tile (partition = x) so the DMA
        # writes are 512B-contiguous runs:
        #   stg2[x] = [C(2s,x) C(2s+1,x) | C(2s+1,x) C(2s+2,x)]
        stg0 = None
        for s in range(NS):
            trp = tpool.tile([128, 512], BF16, tag="tr", name="trp")
            nc.tensor.transpose(trp[0:64, 0:128],
                                x16[:, 2 * s * 64:(2 * s + 1) * 64], identb[:])
            nc.tensor.transpose(trp[0:64, 128:256],
                                x16[:, (2 * s + 1) * 64:(2 * s + 2) * 64],
                                identb[:])
            nc.tensor.transpose(trp[0:64, 256:384],
                                x16[:, (2 * s + 1) * 64:(2 * s + 2) * 64],
                                identb[:])
            if s < NS - 1:
                nc.tensor.transpose(trp[0:64, 384:512],
                                    x16[:, (2 * s + 2) * 64:(2 * s + 3) * 64],
                                    identb[:])
            stg2 = vpool.tile([64, 512], BF16, tag="stg", name="stg")
            nc.scalar.copy(stg2[:], trp[0:64, :])
            if s == 0:
                stg0 = stg2
            eng = nc.sync if s % 2 == 0 else nc.scalar
            if s < NS - 1:
                eng.dma_start(
                    AP(xt2_d.tensor, ((2 * s + PADG) * GW + PADG) * 256,
                       [[256, 64], [GW * 256, 2], [1, 256]]),
                    AP(stg2[:].tensor, stg2[:].offset,
                       [[512, 64], [256, 2], [1, 256]]),
                )
            else:
                eng.dma_start(
                    AP(xt2_d.tensor, ((2 * s + PADG) * GW + PADG) * 256,
                       [[256, 64], [1, 256]]),
                    stg2[:, 0:256],
                )
                eng.dma_start(
                    AP(xt2_d.tensor, ((2 * s + 1 + PADG) * GW + PADG) * 256,
                       [[256, 64], [1, 128]]),
                    stg2[:, 256:384],
                )
        # row PADG-1 second half = C(0): pad row just below the image
        nc.scalar.dma_start(
            AP(xt2_d.tensor, ((PADG - 1) * GW + PADG) * 256 + 128,
               [[256, 64], [1, 128]]),
            stg0[:, 0:128],
        )

        # ---------- sampling positions -> gather indices (before mask) ----
        _cnt = [0]

        def f(shape=(128, NS, K2), dt=F32, tag=None):
            _cnt[0] += 1
            nm = f"cf{_cnt[0]}"
            return dpool.tile(list(shape), dt, tag=tag or nm, name=nm)

        omT_t = omT[:].tensor
        omT_off = omT[:].offset
        dyT = AP(omT_t, omT_off + 0, [[NS * 27, 128], [27, NS], [2, K2]])
        dxT = AP(omT_t, omT_off + 1, [[NS * 27, 128], [27, NS], [2, K2]])
        mlg = omT[:, :, 18:27]

        ykv = ykc.rearrange("p (s a) -> p s a", a=K2)
        xkv = xkc.rearrange("p (s a) -> p s a", a=K2)
        py = f()
        nc.vector.scalar_tensor_tensor(py[:], dyT, hob, ykv, Alu.add, Alu.add)
        px = f()
        nc.vector.scalar_tensor_tensor(px[:], dxT, wo_r, xkv, Alu.add, Alu.add)

        def floorit(v):
            vi = f(dt=I32, tag="fl_i")
            nc.vector.tensor_copy(vi[:], v[:])
            v0 = f(tag="fl_f")
            nc.vector.tensor_copy(v0[:], vi[:])
            gt = f(tag="fl_gt")
            nc.vector.tensor_tensor(gt[:], v0[:], v[:], Alu.is_gt)
            v0f = f()
            nc.vector.tensor_tensor(v0f[:], v0[:], gt[:], Alu.subtract)
            return v0f

        y0f = floorit(py)
        x0f = floorit(px)
        nc.vector.tensor_scalar(y0f[:], y0f[:], -float(PADG), float(H + 2),
                                Alu.max, Alu.min)
        nc.vector.tensor_scalar(x0f[:], x0f[:], -float(PADG), float(W + 2),
                                Alu.max, Alu.min)

        # row index r = (y0+PADG)*GW + (x0+PADG), written k-major: gKM[p][k][s]
        gAf = f()
        nc.vector.tensor_scalar(gAf[:], y0f[:], float(GW), float(PADG * GW + PADG),
                                Alu.mult, Alu.add)
        gKM = dpool.tile([128, K2, NS], F32, tag="gKM", name="gKM")
        gKM_w = AP(gKM[:].tensor, gKM[:].offset, [[K2 * NS, 128], [1, NS], [NS, K2]])
        nc.vector.tensor_tensor(gKM_w, gAf[:], x0f[:], Alu.add)

        # idx wrap via PE transposes:
        # idxAw[16u'+pp][k*256 + s*8 + u] = gKM[16u+pp][k][s] for all u'
        idxAw = spool.tile([128, K2 * 256], I16)
        t1s = []
        for g in range(3):  # pass 1: [128, 96] -> [96, 128], 96 = 3 taps x 32 s
            trp = opool.tile([128, 512], F32, tag="om", name="omp")
            nc.tensor.transpose(
                trp[0:96, 0:128],
                gKM[:].rearrange("p a b -> p (a b)")[:, 96 * g:96 * (g + 1)],
                ident[:],
            )
            t1 = spool.tile([96, 128], F32, tag=f"t1_{g}")
            nc.scalar.copy(t1[:], trp[0:96, 0:128])
            t1s.append(t1)
        for g in range(3):
            for u4 in range(2):  # pass 2: 4x [96, 16] -> [16, 96] per psum buf
                trp = opool.tile([128, 512], F32, tag="om", name="omp")
                for j in range(4):
                    u = 4 * u4 + j
                    nc.tensor.transpose(
                        trp[0:16, j * 128:j * 128 + 96],
                        t1s[g][:, 16 * u:16 * u + 16],
                        ident[0:96, 0:96],
                    )
                t2 = vpool.tile([16, 512], F32, tag="t2", name="t2")
                nc.scalar.copy(t2[:], trp[0:16, :])
                # scatter (j, k', s) -> col (3g+k')*256 + s*8 + (4*u4+j)
                dst = AP(
                    idxAw[:].tensor,
                    idxAw[:].offset + (3 * g) * 256 + 4 * u4,
                    [[K2 * 256, 16], [1, 4], [256, 3], [8, NS]],
                )
                src = AP(
                    t2[:].tensor, t2[:].offset,
                    [[512, 16], [128, 4], [32, 3], [1, 32]],
                )
                nc.vector.tensor_copy(dst, src)
        for u in range(1, 8):  # replicate to all 8 16-partition groups
            nc.scalar.dma_start(idxAw[16 * u:16 * u + 16, :], idxAw[0:16, :])

        # ---------- softmax mask + bilinear coefficients (overlap gathers) --
        e = f()
        nc.scalar.activation(e[:], mlg, mybir.ActivationFunctionType.Exp)
        ssum = f((128, NS, 1))
        nc.vector.tensor_reduce(ssum[:], e[:], mybir.AxisListType.X, Alu.add)
        rs = f((128, NS, 1))
        nc.vector.reciprocal(rs[:], ssum[:])
        mask = f()
        nc.vector.tensor_tensor(mask[:], e[:], rs[:].to_broadcast([128, NS, K2]),
                                Alu.mult)

        wy1 = f()
        nc.vector.tensor_tensor(wy1[:], py[:], y0f[:], Alu.subtract)
        wy0 = f()
        nc.vector.tensor_scalar(wy0[:], wy1[:], -1.0, 1.0, Alu.mult, Alu.add)
        wx1 = f()
        nc.vector.tensor_tensor(wx1[:], px[:], x0f[:], Alu.subtract)
        wx0 = f()
        nc.vector.tensor_scalar(wx0[:], wx1[:], -1.0, 1.0, Alu.mult, Alu.add)

        mwy0 = f()
        nc.vector.tensor_tensor(mwy0[:], mask[:], wy0[:], Alu.mult)
        mwy1 = f()
        nc.vector.tensor_tensor(mwy1[:], mask[:], wy1[:], Alu.mult)
        c00 = f()
        nc.vector.tensor_tensor(c00[:], mwy0[:], wx0[:], Alu.mult)
        c01 = f()
        nc.vector.tensor_tensor(c01[:], mwy0[:], wx1[:], Alu.mult)
        c10 = f()
        nc.vector.tensor_tensor(c10[:], mwy1[:], wx0[:], Alu.mult)
        c11 = f()
        nc.vector.tensor_tensor(c11[:], mwy1[:], wx1[:], Alu.mult)

        # wy1-py etc. consume py/px AFTER idx path; mask ops overlap gathers

        # ---------- main loop ----------
        out_sb = spool.tile([128, HW], F32)
        xt2_src = AP(xt2_d.tensor, 0, [[256, GROWS - 1], [1, 512]])
        for h in range(NHALF):
            outp = ppool.tile([128, PPH], F32, tag="out", name="outp")
            for k in range(K2):
                gb = gpool.tile([128, SPH, 512], BF16, tag="gb", name="gb")
                nc.gpsimd.dma_gather(
                    gb[:], xt2_src,
                    idxAw[:, k * 256 + 128 * h: k * 256 + 128 * h + 128],
                    PPH, PPH, 512, elem_step=256, single_packet=SINGLE_PACKET,
                )
                # corners: [0:128]=A0(c00) [128:256]=B0(c10)
                #          [256:384]=A1(c01) [384:512]=B1(c11)
                mb = vpool.tile([128, SPH, 128], BF16, tag="mb", name="mb")
                for t in range(SPH):
                    s = h * SPH + t
                    nc.scalar.activation(mb[:, t, :], gb[:, t, 0:128], IDENT,
                                         bias=0.0, scale=c00[:, s, k:k + 1])
                hs = h * SPH
                u1 = vpool.tile([128, SPH, 128], BF16, tag="u1", name="u1")
                nc.vector.tensor_tensor(
                    u1[:], gb[:, :, 256:384],
                    c01[:, hs:hs + SPH, k:k + 1].to_broadcast([128, SPH, 128]),
                    Alu.mult)
                u2 = vpool.tile([128, SPH, 128], BF16, tag="u2", name="u2")
                nc.vector.tensor_tensor(
                    u2[:], gb[:, :, 128:256],
                    c10[:, hs:hs + SPH, k:k + 1].to_broadcast([128, SPH, 128]),
                    Alu.mult)
                u3 = vpool.tile([128, SPH, 128], BF16, tag="u3", name="u3")
                nc.vector.tensor_tensor(
                    u3[:], gb[:, :, 384:512],
                    c11[:, hs:hs + SPH, k:k + 1].to_broadcast([128, SPH, 128]),
                    Alu.mult)
                vb = vpool.tile([128, SPH, 128], BF16, tag="vb", name="vb")
                nc.vector.tensor_tensor(vb[:], u1[:], mb[:], Alu.add)
                nc.vector.tensor_tensor(vb[:], vb[:], u2[:], Alu.add)
                nc.vector.tensor_tensor(vb[:], vb[:], u3[:], Alu.add)

                trp = None
                for t in range(SPH):
                    if t % 4 == 0:
                        trp = tpool.tile([128, 512], BF16, tag="tr", name="trp")
                    nc.tensor.transpose(trp[:, (t % 4) * 128:(t % 4) * 128 + 128],
                                        vb[:, t, :], identb[:])
                    if t % 4 == 3:
                        vT = vpool.tile([128, 512], BF16, tag="vT", name="vT")
                        nc.scalar.copy(vT[:], trp[:])
                        bk = t // 4
                        nc.tensor.matmul(
                            outp[:, bk * 512:(bk + 1) * 512], wT[:, k, :], vT[:],
                            start=(k == 0), stop=(k == K2 - 1),
                        )
            for bk in range(4):
                nc.scalar.activation(
                    out_sb[:, h * PPH + bk * 512: h * PPH + (bk + 1) * 512],
                    outp[:, bk * 512:(bk + 1) * 512],
                    IDENT, bias=bias_sb[:], scale=1.0,
                )
            nc.sync.dma_start(
                AP(out_d.tensor, h * PPH, [[HW, 128], [1, PPH]]),
                out_sb[:, h * PPH:(h + 1) * PPH],
            )


def _make_consts():
    c = np.zeros((128, 707), np.float32)
    c[:, 0:128] = np.eye(128, dtype=np.float32)
    p = np.arange(128)
    c[:, 128] = p
    c[:, 129] = (p >= 64)
    c[:, 130] = p % 64
    s = np.arange(32)[:, None, None]
    kyv = np.arange(3)[None, :, None]
    kxv = np.arange(3)[None, None, :]
    c[:, 131:419] = np.broadcast_to(
        (2 * s + kyv - 1 + 0 * kxv).reshape(-1), (128, 288))
    c[:, 419:707] = np.broadcast_to(
        (0 * s + 0 * kyv + kxv - 1).reshape(-1), (128, 288))
    return c


_COMPILED = None


def _get_compiled():
    global _COMPILED
    if _COMPILED is None:
        nc = bacc.Bacc(get_trn_type() or "TRN2", target_bir_lowering=False,
                       debug=False, num_devices=B)
        with tile.TileContext(nc) as tc:
            _emit(tc)
        nc.compile()
        _COMPILED = nc
    return _COMPILED


def kernel(x, w_om, b_om, weight, bias):
    global LAST_EXEC_TIME_NS, LAST_RESULT
    x = np.ascontiguousarray(np.asarray(x, dtype=np.float32))
    w_om_f = np.ascontiguousarray(np.asarray(w_om, np.float32).reshape(27, 1152))
    b_om_f = np.ascontiguousarray(np.asarray(b_om, np.float32).reshape(27, 1))
    weight_f = np.ascontiguousarray(np.asarray(weight, np.float32).reshape(128, 1152))
    bias_f = np.ascontiguousarray(np.asarray(bias, np.float32).reshape(128, 1))

    nc = _get_compiled()
    consts = _make_consts()
    in_maps = [
        {
            "x": np.ascontiguousarray(x[b].reshape(C, HW)),
            "w_om": w_om_f,
            "b_om": b_om_f,
            "weight": weight_f,
            "bias": bias_f,
            "consts": consts,
        }
        for b in range(B)
    ]
    trace = bool(os.environ.get("DCN_TRACE"))
    res = run_bass_kernel_spmd(nc, in_maps, core_ids=list(range(B)), trace=trace)
    LAST_RESULT = res
    LAST_EXEC_TIME_NS = res.exec_time_ns
    out = np.stack([res.results[b]["out"].reshape(C, H, W) for b in range(B)])
    return out.astype(np.float32)


# revision 9
# speedup vs baseline: 3.3055x; 1.2776x over previous
"""DCNv2 (modulated deformable convolution) on 8 Trainium2 NeuronCores.

kernel(**inputs) takes the full unsharded inputs
    x      (8, 128, 64, 64) f32
    w_om   (27, 128, 3, 3)  f32
    b_om   (27,)            f32
    weight (128, 128, 3, 3) f32
    bias   (128,)           f32
and returns the full output (8, 128, 64, 64) f32.

Sharding: pure data-parallel over batch - one image per NeuronCore, small
weights replicated; no collectives.

v3 per-core program (bf16 datapath):
  1. x/weights are cast to bf16 during the load DMA (SWDGE); x is staged
     twice into a DRAM image xt2[GROWS, 256] where row r=(y,x) holds
     [C(y,x), C(y+1,x)] - the 4 bilinear corners of any sample are 4*128
     CONTIGUOUS bf16 values (one 1KB gather descriptor per (pixel, tap)).
     Staging writes are split across the sync and scalar HWDGE rings so
     they don't serialize behind each other.
  2. offset conv (27ch 3x3) on the PE in bf16; sampling positions + gather
     row indices are computed FIRST (the softmax mask / bilinear
     coefficients follow and overlap the first gathers); indices are
     wrapped into the 16-partition dma_gather layout with PE transposes.
  3. per (half, tap) one dma_gather fetches [A0 B0 A1 B1] corner blocks in
     (pixel-partition, channel) layout; corners are combined at
     whole-gather granularity: ACT applies c00 per pixel-tile (16 ops into
     one [128,16,128] tile), DVE does 3 broadcast-coefficient multiplies +
     3 bf16 adds; the result is PE-transposed back to (channel, pixel)
     (4 tiles per PSUM bank) and accumulated over the 9 taps into PSUM
     with the 128x128x3x3 weight; bias is added on the PSUM->SBUF copy.
"""

import os
import sys

import numpy as np

sys.path.insert(0, "/opt/trn_rl_repo")

from contextlib import ExitStack

import concourse.bacc as bacc
import concourse.mybir as mybir
import concourse.tile as tile
from concourse._compat import get_trn_type
from concourse.alu_op_type import AluOpType as Alu
from concourse.bass import AP
from concourse.bass_utils import run_bass_kernel_spmd
from concourse import library_config

F32 = mybir.dt.float32
BF16 = mybir.dt.bfloat16
I32 = mybir.dt.int32
I16 = mybir.dt.int16

B = 8
C = 128
H = W = 64
HW = H * W
K2 = 9
PADG = 4
GW = H + 2 * PADG      # 72
GROWS = GW * GW        # 5184
NS = 32
NHALF = 2
SPH = NS // NHALF      # 16 s-tiles per half
PPH = HW // NHALF      # 2048 pixels per half
IDENT = mybir.ActivationFunctionType.Identity

LAST_EXEC_TIME_NS = None
LAST_RESULT = None
SINGLE_PACKET = bool(int(os.environ.get("DCN_SP", "0")))


def _emit(tc):
    nc = tc.nc
    x_d = nc.dram_tensor("x", [C, HW], F32, kind="ExternalInput").ap()
    w_om_d = nc.dram_tensor("w_om", [27, 1152], F32, kind="ExternalInput").ap()
    b_om_d = nc.dram_tensor("b_om", [27, 1], F32, kind="ExternalInput").ap()
    weight_d = nc.dram_tensor("weight", [C, 1152], F32, kind="ExternalInput").ap()
    bias_d = nc.dram_tensor("bias", [C, 1], F32, kind="ExternalInput").ap()
    out_d = nc.dram_tensor("out", [C, HW], F32, kind="ExternalOutput").ap()
    xt2_d = nc.dram_tensor("xt2_pad", [GROWS, 256], BF16, kind="Internal").ap()
    consts_d = nc.dram_tensor("consts", [128, 707], F32, kind="ExternalInput").ap()

    ctx = ExitStack()
    with ctx:
        cpool = ctx.enter_context(tc.tile_pool(name="const", bufs=1))
        spool = ctx.enter_context(tc.tile_pool(name="setup", bufs=1))
        dpool = ctx.enter_context(tc.tile_pool(name="data", bufs=1))
        gpool = ctx.enter_context(tc.tile_pool(name="gath", bufs=4))
        vpool = ctx.enter_context(tc.tile_pool(name="val", bufs=2))
        ppool = ctx.enter_context(tc.tile_pool(name="psum", bufs=1, space="PSUM"))
        tpool = ctx.enter_context(tc.tile_pool(name="trps", bufs=2, space="PSUM"))
        opool = ctx.enter_context(tc.tile_pool(name="omps", bufs=2, space="PSUM"))

        # ---------- loads (bf16 casts during DMA on the SWDGE ring; small
        # f32 tensors on the sync HWDGE ring, queued before staging) ----------
        cons = cpool.tile([128, 707], F32)
        nc.sync.dma_start(cons[:], consts_d[:, :])
        ident = cons[:, 0:128]
        hob = cons[:, 129:130]
        wo_r = cons[:, 130:131]
        ykc = cons[:, 131:419]
        xkc = cons[:, 419:707]

        x16 = spool.tile([128, HW], BF16)
        nc.gpsimd.dma_start(x16[:], x_d[:, :])
        w16 = spool.tile([128, 1152], BF16)
        nc.gpsimd.dma_start(w16[:], weight_d[:, :])
        w_om16 = spool.tile([27, 1152], BF16)
        nc.gpsimd.dma_start(w_om16[:], w_om_d[:, :])
        nc.gpsimd.load_library(library_config.mlp)

        b_om_sb = spool.tile([27, 1], F32)
        nc.sync.dma_start(b_om_sb[:], b_om_d[:, :])
        bias_sb = spool.tile([128, 1], F32)
        nc.sync.dma_start(bias_sb[:], bias_d[:, :])

        identb = spool.tile([128, 128], BF16)
        nc.vector.tensor_copy(identb[:], ident)

        # ---------- x_pad (bf16) for the offset conv ----------
        XP = 66
        x_pad = spool.tile([128, XP * XP], BF16)
        nc.vector.memset(x_pad[:], 0.0)
        nc.vector.tensor_copy(
            x_pad[:].rearrange("p (a b) -> p a b", a=XP)[:, 1:65, 1:65],
            x16[:].rearrange("p (a b) -> p a b", a=64),
        )

        # ---------- weight transposes ----------
        womT = spool.tile([128, K2, 27], BF16)
        for k in range(K2):
            trp = tpool.tile([128, 512], BF16, tag="tr", name="trp")
            nc.tensor.transpose(
                trp[:, 0:27],
                w_om16[:].rearrange("p (c k) -> p c k", k=K2)[:, :, k],
                identb[0:27, 0:27],
            )
            nc.scalar.copy(womT[:, k, :], trp[:, 0:27])
        wT = spool.tile([128, K2, 128], BF16)
        for k in range(K2):
            trp = tpool.tile([128, 512], BF16, tag="tr", name="trp")
            nc.tensor.transpose(
                trp[:, 0:128],
                w16[:].rearrange("p (c k) -> p c k", k=K2)[:, :, k], identb[:],
            )
            nc.scalar.copy(wT[:, k, :], trp[:, 0:128])

        # ---------- offset conv: om (27, 4096) f32 ----------
        om_sb = spool.tile([27, HW], BF16)
        xpv = x_pad[:].rearrange("p (a b) -> p a b", a=XP)
        for ch in range(8):
            omp = opool.tile([128, 512], F32, tag="om", name="omp")
            for k in range(K2):
                dy_, dx_ = k // 3, k % 3
                r0 = ch * 8 + dy_
                nc.tensor.matmul(
                    omp[0:27, :], womT[:, k, :], xpv[:, r0:r0 + 8, dx_:dx_ + 64],
                    start=(k == 0), stop=(k == K2 - 1),
                )
            nc.scalar.activation(
                om_sb[:, ch * 512:(ch + 1) * 512], omp[0:27, :],
                IDENT, bias=b_om_sb[:], scale=1.0,
            )

        # ---------- omT (128 pix, 27) per s-tile; 4 tiles per psum copy ----
        omT = spool.tile([128, NS, 27], F32)
        for s4 in range(NS // 4):
            trp = tpool.tile([128, 512], BF16, tag="tr", name="trp")
            for j in range(4):
                nc.tensor.transpose(
                    trp[:, j * 128:j * 128 + 27],
                    om_sb[:, (4 * s4 + j) * 128:(4 * s4 + j + 1) * 128],
                    identb[0:27, 0:27],
                )
            nc.scalar.copy(
                omT[:, 4 * s4:4 * s4 + 4, :],
                trp[:].rearrange("p (a b) -> p a b", b=128)[:, :, 0:27],
            )

        # ---------- zero-fill xt2 on the scalar HWDGE ring ----------
        zt = spool.tile([128, 1296], BF16)
        nc.vector.memset(zt[:], 0.0)
        for i in range(8):
            nc.scalar.dma_start(
                AP(xt2_d.tensor, i * 128 * 1296, [[1296, 128], [1, 1296]]), zt[:]
            )

        # ---------- stage xt2 ----------
        # xt2 row (y+PADG, x) = [C(y,x), C(y+1,x)].  Per s we build the two
        # full rows y=2s,2s+1 as a [64, 512] tile (partition = x) so the DMA
        # writes are 512B-contiguous runs:
        #   stg2[x] = [C(2s,x) C(2s+1,x) | C(2s+1,x) C(2s+2,x)]
        stg0 = None
        for s in range(NS):
            trp = tpool.tile([128, 512], BF16, tag="tr", name="trp")
            nc.tensor.transpose(trp[0:64, 0:128],
                                x16[:, 2 * s * 64:(2 * s + 1) * 64], identb[:])
            nc.tensor.transpose(trp[0:64, 128:256],
                                x16[:, (2 * s + 1) * 64:(2 * s + 2) * 64],
                                identb[:])
            nc.tensor.transpose(trp[0:64, 256:384],
                                x16[:, (2 * s + 1) * 64:(2 * s + 2) * 64],
                                identb[:])
            if s < NS - 1:
                nc.tensor.transpose(trp[0:64, 384:512],
                                    x16[:, (2 * s + 2) * 64:(2 * s + 3) * 64],
                                    identb[:])
            stg2 = vpool.tile([64, 512], BF16, tag="stg", name="stg")
            nc.scalar.copy(stg2[:], trp[0:64, :])
            if s == 0:
                stg0 = stg2
            eng = nc.sync if s % 2 == 0 else nc.scalar
            if s < NS - 1:
                eng.dma_start(
                    AP(xt2_d.tensor, ((2 * s + PADG) * GW + PADG) * 256,
                       [[256, 64], [GW * 256, 2], [1, 256]]),
                    AP(stg2[:].tensor, stg2[:].offset,
                       [[512, 64], [256, 2], [1, 256]]),
                )
            else:
                eng.dma_start(
                    AP(xt2_d.tensor, ((2 * s + PADG) * GW + PADG) * 256,
                       [[256, 64], [1, 256]]),
                    stg2[:, 0:256],
                )
                eng.dma_start(
                    AP(xt2_d.tensor, ((2 * s + 1 + PADG) * GW + PADG) * 256,
                       [[256, 64], [1, 128]]),
                    stg2[:, 256:384],
                )
        # row PADG-1 second half = C(0): pad row just below the image
        nc.scalar.dma_start(
            AP(xt2_d.tensor, ((PADG - 1) * GW + PADG) * 256 + 128,
               [[256, 64], [1, 128]]),
            stg0[:, 0:128],
        )

        # ---------- sampling positions -> gather indices (before mask) ----
        _cnt = [0]

        def f(shape=(128, NS, K2), dt=F32, tag=None):
            _cnt[0] += 1
            nm = f"cf{_cnt[0]}"
            return dpool.tile(list(shape), dt, tag=tag or nm, name=nm)

        omT_t = omT[:].tensor
        omT_off = omT[:].offset
        dyT = AP(omT_t, omT_off + 0, [[NS * 27, 128], [27, NS], [2, K2]])
        dxT = AP(omT_t, omT_off + 1, [[NS * 27, 128], [27, NS], [2, K2]])
        mlg = omT[:, :, 18:27]

        ykv = ykc.rearrange("p (s a) -> p s a", a=K2)
        xkv = xkc.rearrange("p (s a) -> p s a", a=K2)
        py = f()
        nc.vector.scalar_tensor_tensor(py[:], dyT, hob, ykv, Alu.add, Alu.add)
        px = f()
        nc.vector.scalar_tensor_tensor(px[:], dxT, wo_r, xkv, Alu.add, Alu.add)

        def floorit(v):
            vi = f(dt=I32, tag="fl_i")
            nc.vector.tensor_copy(vi[:], v[:])
            v0 = f(tag="fl_f")
            nc.vector.tensor_copy(v0[:], vi[:])
            gt = f(tag="fl_gt")
            nc.vector.tensor_tensor(gt[:], v0[:], v[:], Alu.is_gt)
            v0f = f()
            nc.vector.tensor_tensor(v0f[:], v0[:], gt[:], Alu.subtract)
            return v0f

        y0f = floorit(py)
        x0f = floorit(px)
        nc.vector.tensor_scalar(y0f[:], y0f[:], -float(PADG), float(H + 2),
                                Alu.max, Alu.min)
        nc.vector.tensor_scalar(x0f[:], x0f[:], -float(PADG), float(W + 2),
                                Alu.max, Alu.min)

        # row index r = (y0+PADG)*GW + (x0+PADG), written k-major: gKM[p][k][s]
        gAf = f()
        nc.vector.tensor_scalar(gAf[:], y0f[:], float(GW), float(PADG * GW + PADG),
                                Alu.mult, Alu.add)
        gKM = dpool.tile([128, K2, NS], F32, tag="gKM", name="gKM")
        gKM_w = AP(gKM[:].tensor, gKM[:].offset, [[K2 * NS, 128], [1, NS], [NS, K2]])
        nc.vector.tensor_tensor(gKM_w, gAf[:], x0f[:], Alu.add)

        # idx wrap via PE transposes:
        # idxAw[16u'+pp][k*256 + s*8 + u] = gKM[16u+pp][k][s] for all u'
        idxAw = spool.tile([128, K2 * 256], I16)
        t1s = []
        for g in range(3):  # pass 1: [128, 96] -> [96, 128], 96 = 3 taps x 32 s
            trp = opool.tile([128, 512], F32, tag="om", name="omp")
            nc.tensor.transpose(
                trp[0:96, 0:128],
                gKM[:].rearrange("p a b -> p (a b)")[:, 96 * g:96 * (g + 1)],
                ident[:],
            )
            t1 = spool.tile([96, 128], F32, tag=f"t1_{g}")
            nc.scalar.copy(t1[:], trp[0:96, 0:128])
            t1s.append(t1)
        for g in range(3):
            for u4 in range(2):  # pass 2: 4x [96, 16] -> [16, 96] per psum buf
                trp = opool.tile([128, 512], F32, tag="om", name="omp")
                for j in range(4):
                    u = 4 * u4 + j
                    nc.tensor.transpose(
                        trp[0:16, j * 128:j * 128 + 96],
                        t1s[g][:, 16 * u:16 * u + 16],
                        ident[0:96, 0:96],
                    )
                t2 = vpool.tile([16, 512], F32, tag="t2", name="t2")
                nc.scalar.copy(t2[:], trp[0:16, :])
                # scatter (j, k', s) -> col (3g+k')*256 + s*8 + (4*u4+j)
                dst = AP(
                    idxAw[:].tensor,
                    idxAw[:].offset + (3 * g) * 256 + 4 * u4,
                    [[K2 * 256, 16], [1, 4], [256, 3], [8, NS]],
                )
                src = AP(
                    t2[:].tensor, t2[:].offset,
                    [[512, 16], [128, 4], [32, 3], [1, 32]],
                )
                nc.vector.tensor_copy(dst, src)
        for u in range(1, 8):  # replicate to all 8 16-partition groups
            nc.scalar.dma_start(idxAw[16 * u:16 * u + 16, :], idxAw[0:16, :])

        # ---------- softmax mask + bilinear coefficients (overlap gathers) --
        e = f()
        nc.scalar.activation(e[:], mlg, mybir.ActivationFunctionType.Exp)
        ssum = f((128, NS, 1))
        nc.vector.tensor_reduce(ssum[:], e[:], mybir.AxisListType.X, Alu.add)
        rs = f((128, NS, 1))
        nc.vector.reciprocal(rs[:], ssum[:])
        mask = f()
        nc.vector.tensor_tensor(mask[:], e[:], rs[:].to_broadcast([128, NS, K2]),
                                Alu.mult)

        wy1 = f()
        nc.vector.tensor_tensor(wy1[:], py[:], y0f[:], Alu.subtract)
        wy0 = f()
        nc.vector.tensor_scalar(wy0[:], wy1[:], -1.0, 1.0, Alu.mult, Alu.add)
        wx1 = f()
        nc.vector.tensor_tensor(wx1[:], px[:], x0f[:], Alu.subtract)
        wx0 = f()
        nc.vector.tensor_scalar(wx0[:], wx1[:], -1.0, 1.0, Alu.mult, Alu.add)

        mwy0 = f()
        nc.vector.tensor_tensor(mwy0[:], mask[:], wy0[:], Alu.mult)
        mwy1 = f()
        nc.vector.tensor_tensor(mwy1[:], mask[:], wy1[:], Alu.mult)
        c00 = f()
        nc.vector.tensor_tensor(c00[:], mwy0[:], wx0[:], Alu.mult)
        c01 = f()
        nc.vector.tensor_tensor(c01[:], mwy0[:], wx1[:], Alu.mult)
        c10 = f()
        nc.vector.tensor_tensor(c10[:], mwy1[:], wx0[:], Alu.mult)
        c11 = f()
        nc.vector.tensor_tensor(c11[:], mwy1[:], wx1[:], Alu.mult)

        # wy1-py etc. consume py/px AFTER idx path; mask ops overlap gathers

        # ---------- main loop ----------
        out_sb = spool.tile([128, HW], F32)
        xt2_src = AP(xt2_d.tensor, 0, [[256, GROWS - 1], [1, 512]])
        for h in range(NHALF):
            outp = ppool.tile([128, PPH], F32, tag="out", name="outp")
            for k in range(K2):
                gb = gpool.tile([128, SPH, 512], BF16, tag="gb", name="gb")
                nc.gpsimd.dma_gather(
                    gb[:], xt2_src,
                    idxAw[:, k * 256 + 128 * h: k * 256 + 128 * h + 128],
                    PPH, PPH, 512, elem_step=256, single_packet=SINGLE_PACKET,
                    queue_num=(h * K2 + k) % 4,
                )
                # corners: [0:128]=A0(c00) [128:256]=B0(c10)
                #          [256:384]=A1(c01) [384:512]=B1(c11)
                mb = vpool.tile([128, SPH, 128], BF16, tag="mb", name="mb")
                for t in range(SPH):
                    s = h * SPH + t
                    nc.scalar.activation(mb[:, t, :], gb[:, t, 0:128], IDENT,
                                         bias=0.0, scale=c00[:, s, k:k + 1])
                hs = h * SPH
                u1 = vpool.tile([128, SPH, 128], BF16, tag="u1", name="u1")
                nc.vector.tensor_tensor(
                    u1[:], gb[:, :, 256:384],
                    c01[:, hs:hs + SPH, k:k + 1].to_broadcast([128, SPH, 128]),
                    Alu.mult)
                u2 = vpool.tile([128, SPH, 128], BF16, tag="u2", name="u2")
                nc.vector.tensor_tensor(
                    u2[:], gb[:, :, 128:256],
                    c10[:, hs:hs + SPH, k:k + 1].to_broadcast([128, SPH, 128]),
                    Alu.mult)
                u3 = vpool.tile([128, SPH, 128], BF16, tag="u3", name="u3")
                nc.vector.tensor_tensor(
                    u3[:], gb[:, :, 384:512],
                    c11[:, hs:hs + SPH, k:k + 1].to_broadcast([128, SPH, 128]),
                    Alu.mult)
                vb = vpool.tile([128, SPH, 128], BF16, tag="vb", name="vb")
                nc.vector.tensor_tensor(vb[:], u1[:], mb[:], Alu.add)
                nc.vector.tensor_tensor(vb[:], vb[:], u2[:], Alu.add)
                nc.vector.tensor_tensor(vb[:], vb[:], u3[:], Alu.add)

                trp = None
                for t in range(SPH):
                    if t % 4 == 0:
                        trp = tpool.tile([128, 512], BF16, tag="tr", name="trp")
                    nc.tensor.transpose(trp[:, (t % 4) * 128:(t % 4) * 128 + 128],
                                        vb[:, t, :], identb[:])
                    if t % 4 == 3:
                        vT = vpool.tile([128, 512], BF16, tag="vT", name="vT")
                        nc.scalar.copy(vT[:], trp[:])
                        bk = t // 4
                        nc.tensor.matmul(
                            outp[:, bk * 512:(bk + 1) * 512], wT[:, k, :], vT[:],
                            start=(k == 0), stop=(k == K2 - 1),
                        )
            for bk in range(4):
                nc.scalar.activation(
                    out_sb[:, h * PPH + bk * 512: h * PPH + (bk + 1) * 512],
                    outp[:, bk * 512:(bk + 1) * 512],
                    IDENT, bias=bias_sb[:], scale=1.0,
                )
            nc.sync.dma_start(
                AP(out_d.tensor, h * PPH, [[HW, 128], [1, PPH]]),
                out_sb[:, h * PPH:(h + 1) * PPH],
            )


def _make_consts():
    c = np.zeros((128, 707), np.float32)
    c[:, 0:128] = np.eye(128, dtype=np.float32)
    p = np.arange(128)
    c[:, 128] = p
    c[:, 129] = (p >= 64)
    c[:, 130] = p % 64
    s = np.arange(32)[:, None, None]
    kyv = np.arange(3)[None, :, None]
    kxv = np.arange(3)[None, None, :]
    c[:, 131:419] = np.broadcast_to(
        (2 * s + kyv - 1 + 0 * kxv).reshape(-1), (128, 288))
    c[:, 419:707] = np.broadcast_to(
        (0 * s + 0 * kyv + kxv - 1).reshape(-1), (128, 288))
    return c


_COMPILED = None


def _get_compiled():
    global _COMPILED
    if _COMPILED is None:
        nc = bacc.Bacc(get_trn_type() or "TRN2", target_bir_lowering=False,
                       debug=False, num_devices=B, num_swdge_queues=4)
        with tile.TileContext(nc) as tc:
            _emit(tc)
        nc.compile()
        _COMPILED = nc
    return _COMPILED


def kernel(x, w_om, b_om, weight, bias):
    global LAST_EXEC_TIME_NS, LAST_RESULT
    x = np.ascontiguousarray(np.asarray(x, dtype=np.float32))
    w_om_f = np.ascontiguousarray(np.asarray(w_om, np.float32).reshape(27, 1152))
    b_om_f = np.ascontiguousarray(np.asarray(b_om, np.float32).reshape(27, 1))
    weight_f = np.ascontiguousarray(np.asarray(weight, np.float32).reshape(128, 1152))
    bias_f = np.ascontiguousarray(np.asarray(bias, np.float32).reshape(128, 1))

    nc = _get_compiled()
    consts = _make_consts()
    in_maps = [
        {
            "x": np.ascontiguousarray(x[b].reshape(C, HW)),
            "w_om": w_om_f,
            "b_om": b_om_f,
            "weight": weight_f,
            "bias": bias_f,
            "consts": consts,
        }
        for b in range(B)
    ]
    trace = bool(os.environ.get("DCN_TRACE"))
    res = run_bass_kernel_spmd(nc, in_maps, core_ids=list(range(B)), trace=trace)
    LAST_RESULT = res
    LAST_EXEC_TIME_NS = res.exec_time_ns
    out = np.stack([res.results[b]["out"].reshape(C, H, W) for b in range(B)])
    return out.astype(np.float32)
